# revision 2
# baseline (speedup 1.0000x reference)
"""Trainium2 Bass kernel for the GNN ExplainModule (masked adjacency).

Strategy (8 NeuronCores, row-sharded output):
  - Each core owns 1250 rows of the [10000, 10000] output, processed in
    row-blocks of 128.
  - Host routes each edge's two contributions ((r,c) and (c,r), weight
    0.5*gate) to the owning core/block, sorted by destination; indices
    only — all FP math runs on device.
  - Device tables via PE: A = (embed @ W1a + 1 x c_vec) * |W2|,
    B = (embed @ W1b) * |W2|  (hidden units permuted so W2 >= 0 first;
    signs re-applied as pos-reduce minus neg-reduce).
  - Per contribution: dma_gather A[row], B[col] and the 64-wide adj
    segment holding (r, c); compute gate = sigmoid(logit(noise) + mlp);
    payload = onehot64(c % 64) * adj_seg * (0.5 * gate * valid);
    dma_scatter_add payload into the output (CCE add; duplicate dests
    accumulate natively; output buffers arrive pre-zeroed via PJRT
    donation so untouched cells stay 0).
"""

import sys

import numpy as np

for _p in ("/opt/trn_rl_repo",):
    if _p not in sys.path:
        sys.path.insert(0, _p)

N = 10000
D = 64
NCORES = 8
RPC = N // NCORES  # rows per core
BLK = 128  # rows per block
SEG = -(-N // 64)  # 64-wide segments per row (157)
SEGX = SEG + 1  # +1 pad segment per row (scatter pad target)
PITCH = SEGX * 64  # padded row pitch
SUB = 1024  # tokens per custom-DMA op


def _blocks():
    out = []
    r = 0
    while r < RPC:
        h = min(BLK, RPC - r)
        out.append((r, h))
        r += h
    return out


def _prep_host(row, col, noise):
    """Route contributions to (core, block); build packed token arrays."""
    row = np.asarray(row).astype(np.int64).ravel()
    col = np.asarray(col).astype(np.int64).ravel()
    noise = np.asarray(noise).astype(np.float32).ravel()

    dr = np.concatenate([row, col])  # dest row
    dc = np.concatenate([col, row])  # dest col
    ea = np.concatenate([row, row])  # A-table index
    eb = np.concatenate([col, col])  # B-table index
    en = np.concatenate([noise, noise])
    core = dr // RPC

    blocks = _blocks()
    nblk = len(blocks)
    # per core, per block, per wave: token arrays. A scatter instruction must
    # not carry two tokens targeting the same 64-wide segment row (the HW CCE
    # adds race within one instruction); the w-th token of each segment group
    # goes to wave w, and waves scatter in separate, serialized instructions.
    toks = [[None] * nblk for _ in range(NCORES)]
    n_waves = 1
    for k in range(NCORES):
        m = core == k
        rl = dr[m] - k * RPC
        d = rl * N + dc[m]
        o = np.argsort(d, kind="stable")
        rl, dcc, a, b, nz = rl[o], dc[m][o], ea[m][o], eb[m][o], en[m][o]
        blk_id = rl // BLK
        for bi, (r0, h) in enumerate(blocks):
            sel = blk_id == bi
            si = (rl[sel] - r0) * SEGX + dcc[sel] // 64
            # occurrence rank of each token within its segment group (tokens
            # are sorted by dest, so equal si values are adjacent)
            uq, inv, cnt = np.unique(si, return_inverse=True, return_counts=True)
            starts = np.zeros(len(uq) + 1, np.int64)
            np.cumsum(cnt, out=starts[1:])
            rank = np.arange(len(si)) - starts[inv]
            n_waves = max(n_waves, int(cnt.max()) if len(cnt) else 1)
            toks[k][bi] = (
                a[sel],
                b[sel],
                nz[sel],
                si,
                (dcc[sel] % 64).astype(np.float32),
                rank,
            )

    # SPMD-static chunk sizes per (block, wave)
    chunk_list = []  # (block_idx, row0, blk_h, t, off16, off128)
    key_sizes = {}  # (bi, w) -> padded size
    off16 = off128 = 0
    for bi, (r0, h) in enumerate(blocks):
        for w in range(n_waves):
            t_bw = max(
                int((toks[k][bi][5] == w).sum()) for k in range(NCORES)
            )
            if w == 0:
                t_bw = max(t_bw, 1)
            if t_bw == 0:
                continue
            t_bw = -(-t_bw // 128) * 128
            key_sizes[(bi, w)] = t_bw
            done = 0
            while done < t_bw:
                t = min(SUB, t_bw - done)
                chunk_list.append((bi, r0, h, t, off16, off128))
                off16 += t // 16
                off128 += t // 128
                done += t
    total16, total128 = off16, off128

    pad_si = SEGX - 1  # row 0's pad segment; never holds real data

    per_core = []
    for k in range(NCORES):
        ga16 = np.zeros((128, total16), np.int16)
        gb16 = np.zeros((128, total16), np.int16)
        si16 = np.full((128, total16), 0, np.int16)
        nzf = np.full((128, total128), 0.5, np.float32)
        cmf = np.zeros((128, total128), np.float32)
        vmf = np.zeros((128, total128), np.float32)
        ci = 0
        for bi, (r0, h) in enumerate(blocks):
            a0, b0, nz0, si0, cm0, rank0 = toks[k][bi]
            for w in range(n_waves):
                if (bi, w) not in key_sizes:
                    continue
                t_bw = key_sizes[(bi, w)]
                sel = rank0 == w
                n = int(sel.sum())
                pad = t_bw - n
                a = np.concatenate([a0[sel], np.zeros(pad, np.int64)])
                b = np.concatenate([b0[sel], np.zeros(pad, np.int64)])
                nz = np.concatenate([nz0[sel], np.full(pad, 0.5, np.float32)])
                si = np.concatenate([si0[sel], np.full(pad, pad_si, np.int64)])
                cm = np.concatenate([cm0[sel], np.zeros(pad, np.float32)])
                vm = np.concatenate(
                    [np.ones(n, np.float32), np.zeros(pad, np.float32)]
                )
                done = 0
                while done < t_bw:
                    bi2, _r0, _h, t, o16, o128 = chunk_list[ci]
                    assert bi2 == bi and done + t <= t_bw
                    sl = slice(done, done + t)

                    def wrap16(x):
                        return np.tile(
                            np.ascontiguousarray(x[sl].reshape(-1, 16).T),
                            (8, 1),
                        )

                    def wrap128(x):
                        return np.ascontiguousarray(x[sl].reshape(-1, 128).T)

                    ga16[:, o16 : o16 + t // 16] = wrap16(a).astype(np.int16)
                    gb16[:, o16 : o16 + t // 16] = wrap16(b).astype(np.int16)
                    si16[:, o16 : o16 + t // 16] = wrap16(si).astype(np.int16)
                    nzf[:, o128 : o128 + t // 128] = wrap128(nz)
                    cmf[:, o128 : o128 + t // 128] = wrap128(cm)
                    vmf[:, o128 : o128 + t // 128] = wrap128(vm)
                    done += t
                    ci += 1
        assert ci == len(chunk_list)
        per_core.append(
            dict(ga16=ga16, gb16=gb16, si16=si16, nz=nzf, cm=cmf, vm=vmf)
        )
    return per_core, chunk_list, total16, total128


def _build_program(chunk_list, total16, total128, node_idx, b2f, pos_cnt):
    import concourse.bacc as bacc
    import concourse.bass as bass
    import concourse.mybir as mybir
    import concourse.tile as tile
    from concourse.masks import make_identity

    f32 = mybir.dt.float32
    i16 = mybir.dt.int16
    add = mybir.AluOpType.add
    mult = mybir.AluOpType.mult
    subtract = mybir.AluOpType.subtract
    is_equal = mybir.AluOpType.is_equal
    AF = mybir.ActivationFunctionType

    nc = bacc.Bacc()

    blocks = _blocks()
    out_rows = sum(BLK for _ in blocks)  # padded block heights (128 each)

    embp = nc.declare_dram_parameter("embed", [N, D], f32, isOutput=False)
    w1p = nc.declare_dram_parameter("w1", [3 * D, D], f32, isOutput=False)
    b1p = nc.declare_dram_parameter("b1r", [1, D], f32, isOutput=False)
    w2p = nc.declare_dram_parameter("w2b", [128, D], f32, isOutput=False)
    iop = nc.declare_dram_parameter("iota64", [128, D], f32, isOutput=False)
    adjp = nc.declare_dram_parameter("adjp", [out_rows, PITCH], f32, isOutput=False)
    gap = nc.declare_dram_parameter("ga16", [128, total16], i16, isOutput=False)
    gbp = nc.declare_dram_parameter("gb16", [128, total16], i16, isOutput=False)
    sip = nc.declare_dram_parameter("si16", [128, total16], i16, isOutput=False)
    nzp = nc.declare_dram_parameter("nz", [128, total128], f32, isOutput=False)
    cmp_ = nc.declare_dram_parameter("cm", [128, total128], f32, isOutput=False)
    vmp = nc.declare_dram_parameter("vm", [128, total128], f32, isOutput=False)
    outp = nc.declare_dram_parameter("out", [out_rows, PITCH], f32, isOutput=True)

    a_dram = nc.dram_tensor("a_table", [N, D], f32)
    b_dram = nc.dram_tensor("b_table", [N, D], f32)

    NBLKA = -(-N // 128)

    with tile.TileContext(nc) as tc:
        with (
            tc.tile_pool(name="const", bufs=1) as cp,
            tc.tile_pool(name="stagea", bufs=3) as sp,
            tc.tile_pool(name="work", bufs=2) as wp,
            tc.tile_pool(name="psum", bufs=2, space="PSUM") as pp,
        ):
            identity = cp.tile([128, 128], f32)
            make_identity(nc, identity[:])
            w1a = cp.tile([D, D], f32)
            nc.sync.dma_start(out=w1a[:], in_=w1p[0:D, :])
            w1b = cp.tile([D, D], f32)
            nc.sync.dma_start(out=w1b[:], in_=w1p[D : 2 * D, :])
            w1c = cp.tile([D, D], f32)
            nc.sync.dma_start(out=w1c[:], in_=w1p[2 * D : 3 * D, :])
            b1t = cp.tile([1, D], f32)
            nc.sync.dma_start(out=b1t[:], in_=b1p[:, :])
            w2t = cp.tile([128, D], f32)
            nc.sync.dma_start(out=w2t[:], in_=w2p[:, :])
            iot = cp.tile([128, D], f32)
            nc.sync.dma_start(out=iot[:], in_=iop[:, :])
            ones = cp.tile([1, 128], f32)
            nc.vector.memset(ones[:], 1.0)
            e5 = cp.tile([D, 1], f32)
            nc.sync.dma_start(
                out=e5[:], in_=embp[node_idx : node_idx + 1, :].rearrange("o d -> d o")
            )

            # c_vec = embed[node_idx] @ W1c + b1  -> [1, D]
            cps = pp.tile([1, D], f32, tag="cps")
            nc.tensor.matmul(cps[:], lhsT=e5[:], rhs=w1c[:], start=True, stop=True)
            crow = cp.tile([1, D], f32)
            nc.vector.tensor_tensor(out=crow[:], in0=cps[:], in1=b1t[:], op=add)

            # Stage A: A = (embed @ W1a + 1 x crow) * |W2| ; B = (embed @ W1b) * |W2|
            for blk in range(NBLKA):
                r0 = blk * 128
                p = min(128, N - r0)
                et = sp.tile([128, D], f32, tag="et")
                nc.sync.dma_start(out=et[:p, :], in_=embp[r0 : r0 + p, :])
                tps = pp.tile([D, 128], f32, tag="tps")
                nc.tensor.transpose(tps[:, :p], et[:p, :], identity[:p, :p])
                tsb = sp.tile([D, 128], f32, tag="tsb")
                nc.scalar.copy(out=tsb[:, :p], in_=tps[:, :p])
                pa_ = pp.tile([128, D], f32, tag="pa")
                nc.tensor.matmul(
                    pa_[:p, :], lhsT=tsb[:, :p], rhs=w1a[:], start=True, stop=False
                )
                nc.tensor.matmul(
                    pa_[:p, :], lhsT=ones[:, :p], rhs=crow[:], start=False, stop=True
                )
                asb = sp.tile([128, D], f32, tag="asb")
                nc.vector.tensor_tensor(
                    out=asb[:p, :], in0=pa_[:p, :], in1=w2t[:p, :], op=mult
                )
                nc.sync.dma_start(out=a_dram[r0 : r0 + p, :], in_=asb[:p, :])
                pb_ = pp.tile([128, D], f32, tag="pb")
                nc.tensor.matmul(
                    pb_[:p, :], lhsT=tsb[:, :p], rhs=w1b[:], start=True, stop=True
                )
                bsb = sp.tile([128, D], f32, tag="bsb")
                nc.vector.tensor_tensor(
                    out=bsb[:p, :], in0=pb_[:p, :], in1=w2t[:p, :], op=mult
                )
                nc.sync.dma_start(out=b_dram[r0 : r0 + p, :], in_=bsb[:p, :])

            # contribution chunks
            for bi, r0b, h, t, o16, o128 in chunk_list:
                S = t // 128
                S16 = t // 16
                gai = wp.tile([128, S16], i16, tag="gai")
                nc.sync.dma_start(out=gai[:], in_=gap[:, o16 : o16 + S16])
                gbi = wp.tile([128, S16], i16, tag="gbi")
                nc.sync.dma_start(out=gbi[:], in_=gbp[:, o16 : o16 + S16])
                sii = wp.tile([128, S16], i16, tag="sii")
                nc.sync.dma_start(out=sii[:], in_=sip[:, o16 : o16 + S16])
                nz = wp.tile([128, S], f32, tag="nz")
                nc.sync.dma_start(out=nz[:], in_=nzp[:, o128 : o128 + S])
                cm = wp.tile([128, S], f32, tag="cm")
                nc.sync.dma_start(out=cm[:], in_=cmp_[:, o128 : o128 + S])
                vm = wp.tile([128, S], f32, tag="vm")
                nc.sync.dma_start(out=vm[:], in_=vmp[:, o128 : o128 + S])

                ga = wp.tile([128, S * D], f32, tag="ga")
                nc.gpsimd.dma_gather(
                    out_ap=ga[:].rearrange("p (s d) -> p s d", d=D),
                    in_ap=a_dram[:, :],
                    idxs_ap=gai[:],
                    num_idxs=t,
                    num_idxs_reg=t,
                    elem_size=D,
                )
                gb = wp.tile([128, S * D], f32, tag="gb")
                nc.gpsimd.dma_gather(
                    out_ap=gb[:].rearrange("p (s d) -> p s d", d=D),
                    in_ap=b_dram[:, :],
                    idxs_ap=gbi[:],
                    num_idxs=t,
                    num_idxs_reg=t,
                    elem_size=D,
                )
                adjseg = wp.tile([128, S * D], f32, tag="adjseg")
                adj_view = adjp[r0b : r0b + BLK, :].rearrange(
                    "p (s w) -> (p s) w", w=64
                )
                nc.gpsimd.dma_gather(
                    out_ap=adjseg[:].rearrange("p (s d) -> p s d", d=D),
                    in_ap=adj_view,
                    idxs_ap=sii[:],
                    num_idxs=t,
                    num_idxs_reg=t,
                    elem_size=D,
                )

                # MLP: pre = ga + gb ; q = relu(pre) ; s = sum_pos - sum_neg
                nc.vector.tensor_tensor(out=ga[:], in0=ga[:], in1=gb[:], op=add)
                nc.scalar.activation(out=ga[:], in_=ga[:], func=AF.Relu)
                q3 = ga[:].rearrange("p (s d) -> p s d", d=D)
                s = wp.tile([128, S], f32, tag="s")
                if pos_cnt == D:
                    nc.vector.tensor_reduce(
                        out=s[:], in_=q3, axis=mybir.AxisListType.X, op=add
                    )
                elif pos_cnt == 0:
                    nc.vector.tensor_reduce(
                        out=s[:], in_=q3, axis=mybir.AxisListType.X, op=add,
                        negate=True,
                    )
                else:
                    nc.vector.tensor_reduce(
                        out=s[:], in_=q3[:, :, :pos_cnt],
                        axis=mybir.AxisListType.X, op=add,
                    )
                    sn = wp.tile([128, S], f32, tag="sn")
                    nc.vector.tensor_reduce(
                        out=sn[:], in_=q3[:, :, pos_cnt:],
                        axis=mybir.AxisListType.X, op=add,
                    )
                    nc.vector.tensor_tensor(
                        out=s[:], in0=s[:], in1=sn[:], op=subtract
                    )

                # gate = sigmoid(ln(nz) - ln(1-nz) + s + b2)
                om = wp.tile([128, S], f32, tag="om")
                nc.vector.tensor_scalar(
                    out=om[:], in0=nz[:], scalar1=-1.0, scalar2=1.0,
                    op0=mult, op1=add,
                )
                ln1 = wp.tile([128, S], f32, tag="ln1")
                nc.scalar.activation(out=ln1[:], in_=nz[:], func=AF.Ln)
                ln2 = wp.tile([128, S], f32, tag="ln2")
                nc.scalar.activation(out=ln2[:], in_=om[:], func=AF.Ln)
                z = wp.tile([128, S], f32, tag="z")
                nc.vector.scalar_tensor_tensor(
                    out=z[:], in0=ln1[:], scalar=b2f, in1=ln2[:],
                    op0=add, op1=subtract,
                )
                nc.vector.tensor_tensor(out=z[:], in0=z[:], in1=s[:], op=add)
                g = wp.tile([128, S], f32, tag="g")
                nc.scalar.activation(out=g[:], in_=z[:], func=AF.Sigmoid)
                gm = wp.tile([128, S], f32, tag="gm")
                nc.vector.scalar_tensor_tensor(
                    out=gm[:], in0=g[:], scalar=0.5, in1=vm[:],
                    op0=mult, op1=mult,
                )

                # payload = onehot(cm) * adjseg * gm
                oh = wp.tile([128, S * D], f32, tag="oh")
                oh3 = oh[:].rearrange("p (s d) -> p s d", d=D)
                io_b = iot[:].rearrange("p (o d) -> p o d", o=1).to_broadcast(
                    [128, S, D]
                )
                cm_b = cm[:].rearrange("p (s o) -> p s o", o=1).to_broadcast(
                    [128, S, D]
                )
                nc.vector.tensor_tensor(out=oh3, in0=io_b, in1=cm_b, op=is_equal)
                nc.vector.tensor_tensor(out=oh[:], in0=oh[:], in1=adjseg[:], op=mult)
                gm_b = gm[:].rearrange("p (s o) -> p s o", o=1).to_broadcast(
                    [128, S, D]
                )
                nc.vector.tensor_tensor(out=oh3, in0=oh3, in1=gm_b, op=mult)

                out_view = outp[r0b : r0b + BLK, :].rearrange(
                    "p (s w) -> (p s) w", w=64
                )
                nc.gpsimd.dma_scatter_add(
                    out_ap=out_view,
                    in_ap=oh[:].rearrange("p (s d) -> p s d", d=D),
                    idxs_ap=sii[:],
                    num_idxs=t,
                    num_idxs_reg=t,
                    elem_size=D,
                )

    nc.compile()
    return nc


def kernel(embed, row, col, adj, noise, W1, b1, W2, b2, node_idx):
    from concourse.bass_utils import run_bass_kernel_spmd

    embed = np.ascontiguousarray(np.asarray(embed), dtype=np.float32)
    adj = np.ascontiguousarray(np.asarray(adj), dtype=np.float32)
    W1 = np.ascontiguousarray(np.asarray(W1), dtype=np.float32)
    b1 = np.ascontiguousarray(np.asarray(b1), dtype=np.float32).ravel()
    W2 = np.ascontiguousarray(np.asarray(W2), dtype=np.float32)
    b2f = float(np.asarray(b2, dtype=np.float32).ravel()[0])
    nidx = int(np.asarray(node_idx))

    # permute hidden units: W2 >= 0 first; fold |W2| on device
    w2v = W2.reshape(-1).astype(np.float32)
    order = np.argsort(w2v < 0, kind="stable")
    pos_cnt = int((w2v >= 0).sum())
    W1p = np.ascontiguousarray(W1[:, order])
    b1p = np.ascontiguousarray(b1[order]).reshape(1, D)
    w2b = np.ascontiguousarray(
        np.tile(np.abs(w2v[order]).reshape(1, D), (128, 1))
    )
    iota64 = np.ascontiguousarray(
        np.tile(np.arange(D, dtype=np.float32).reshape(1, D), (128, 1))
    )

    per_core, chunk_list, total16, total128 = _prep_host(row, col, noise)
    nc = _build_program(chunk_list, total16, total128, nidx, b2f, pos_cnt)

    blocks = _blocks()
    out_rows = BLK * len(blocks)
    in_maps = []
    for k in range(NCORES):
        adjpad = np.zeros((out_rows, PITCH), np.float32)
        sl = adj[k * RPC : (k + 1) * RPC]
        adjpad[: sl.shape[0], :N] = sl
        m = dict(per_core[k])
        m.update(
            embed=embed, w1=W1p, b1r=b1p, w2b=w2b, iota64=iota64, adjp=adjpad
        )
        in_maps.append(m)

    res = run_bass_kernel_spmd(nc, in_maps, list(range(NCORES)))
    kernel.last_exec_time_ns = res.exec_time_ns
    it = getattr(res, "instructions_and_trace", None)
    kernel.last_trace_path = it[1] if it else None
    pieces = []
    for k in range(NCORES):
        o = res.results[k]["out"]
        # blocks are stacked at BLK spacing; real rows of block bi: r0..r0+h
        for bi, (r0, h) in enumerate(blocks):
            pieces.append(o[bi * BLK : bi * BLK + h, :N])
    out = np.concatenate(pieces, axis=0)
    return out


kernel.last_exec_time_ns = None



# revision 5
# speedup vs baseline: 5.8450x; 5.8450x over previous
"""Trainium2 Bass kernel for the GNN ExplainModule (masked adjacency).

Dense row-block design (8 NeuronCores, row-sharded output):
  - Core k owns rows [k*1250, (k+1)*1250). Rows are re-ordered by token
    count and grouped into 10 blocks of 125 rows (partitions 0-124).
  - Every mask contribution ("token") for cell (r, c) lives in the
    partition of its dest row r. Two streams per block: stream1 = copy1
    tokens (dest side uses W1a), stream2 = copy2 (dest side uses W1b).
  - MLP runs in transposed layout [64 hidden x tokens]: host pre-gathers
    embed[c] columns (bf16) per token; PE computes (E @ W1x)^T per
    <=512-token chunk; the dest-row term (Eblk @ W1y + c)^T is added via
    a partition-run broadcast view; relu; PE matvec with signed W2 gives
    per-token logits [1, T]; a DRAM round-trip reshapes [1, 128*S] into
    the fat [128, S] layout where the concrete gate is computed.
  - gpsimd local_scatter turns each block's gates into a dense
    [128, 2000] mask chunk; DVE multiplies by the adj chunk (bf16); the
    product is written out densely. Duplicate cells (same (r,c) fed by
    several edges) keep the first token in the dense path; the rare
    followers (~300/core) are applied afterwards with per-rank
    dma_scatter_add CCE adds of one-hot payloads.
"""

import sys

import numpy as np

for _p in ("/opt/trn_rl_repo",):
    if _p not in sys.path:
        sys.path.insert(0, _p)

import ml_dtypes

BF16 = ml_dtypes.bfloat16

N = 10000
D = 64
NCORES = 8
RPC = N // NCORES  # 1250 rows per core
NBLK = 10
RPB = RPC // NBLK  # 125 real rows per block
COLS = 10240  # padded row pitch (80 segs of 128)
NCHUNK = 5
CHW = 2000  # dense chunk width
SEGW = 128  # scatter-add segment width (bf16 -> 256B)
NSEG = COLS // SEGW  # 80


def _group_rank(key):
    """Rank of each element within its key-group (appearance order)."""
    o = np.argsort(key, kind="stable")
    ks = key[o]
    starts = np.flatnonzero(np.concatenate([[True], ks[1:] != ks[:-1]]))
    sizes = np.diff(np.concatenate([starts, [len(ks)]]))
    rank_sorted = np.arange(len(ks)) - np.repeat(starts, sizes)
    rank = np.empty(len(key), np.int64)
    rank[o] = rank_sorted
    return rank


def _prep_host(row, col, noise, adj, embed):
    """Route tokens, balance rows into blocks, build all per-core arrays."""
    row = np.asarray(row).astype(np.int64).ravel()
    col = np.asarray(col).astype(np.int64).ravel()
    noise = np.asarray(noise).astype(np.float32).ravel()
    adj = np.asarray(adj, dtype=np.float32)
    embed = np.asarray(embed, dtype=np.float32)
    embed_bf = embed.astype(BF16)

    E = row.shape[0]
    t_r = np.concatenate([row, col])  # dest row
    t_c = np.concatenate([col, row])  # dest col == other endpoint
    t_nz = np.concatenate([noise, noise])
    t_st = np.concatenate([np.zeros(E, np.int8), np.ones(E, np.int8)])
    core_of = t_r // RPC

    # ---- pass 1: per-core row stats and block structure ----
    per_core_tok = []
    orders = []
    S1 = np.zeros((NCORES, NBLK), np.int64)
    S2 = np.zeros((NCORES, NBLK), np.int64)
    for k in range(NCORES):
        m = core_of == k
        r_loc = (t_r[m] - k * RPC).astype(np.int64)
        cc = t_c[m].astype(np.int64)
        nz = t_nz[m]
        st = t_st[m].astype(np.int64)
        n1 = np.bincount(r_loc[st == 0], minlength=RPC)
        n2 = np.bincount(r_loc[st == 1], minlength=RPC)
        order = np.argsort(-(n1 + n2), kind="stable")
        orders.append(order)
        blk_of_row = np.empty(RPC, np.int64)
        part_of_row = np.empty(RPC, np.int64)
        for b in range(NBLK):
            rows_b = order[b * RPB : (b + 1) * RPB]
            blk_of_row[rows_b] = b
            part_of_row[rows_b] = np.arange(RPB)
            S1[k, b] = max(int(n1[rows_b].max()), 1)
            S2[k, b] = max(int(n2[rows_b].max()), 1)
        per_core_tok.append((r_loc, cc, nz, st, blk_of_row, part_of_row))

    # SPMD-static shapes: max over cores, SB even
    S1s = S1.max(axis=0)
    S2s = S2.max(axis=0)
    S1s = S1s + (S1s + S2s) % 2
    SB = S1s + S2s

    colb = np.concatenate([[0], np.cumsum(SB)]).astype(np.int64)
    SBT = int(colb[-1])
    o1 = np.concatenate([[0], np.cumsum(128 * S1s)]).astype(np.int64)
    o2 = np.concatenate([[0], np.cumsum(128 * S2s)]).astype(np.int64)
    T1, T2 = int(o1[-1]), int(o2[-1])
    sidx_off = np.concatenate([[0], np.cumsum(NCHUNK * SB)]).astype(np.int64)

    # ---- pass 2: slots, duplicates, fixup ranks ----
    staged = []
    F = np.zeros((NCORES, NBLK), np.int64)
    for k in range(NCORES):
        r_loc, cc, nz, st, blk_of_row, part_of_row = per_core_tok[k]
        b_s = blk_of_row[r_loc]
        p_s = part_of_row[r_loc]
        # sort tokens by (b, p, stream, c) for slot assignment
        key = ((b_s * 128 + p_s) * 2 + st) * N + cc
        o = np.argsort(key, kind="stable")
        b_s, p_s, c_s, nz_s, st_s = b_s[o], p_s[o], cc[o], nz[o], st[o]
        slot = _group_rank((b_s * 128 + p_s) * 2 + st_s)
        fat = np.where(st_s == 0, slot, S1s[b_s] + slot)
        # duplicates: same (b, p, c); leader = first in (stream, slot) order
        crank = _group_rank((b_s * 128 + p_s) * N + c_s)
        is_fol = crank > 0
        frank = np.full(len(b_s), -1, np.int64)
        fi = np.flatnonzero(is_fol)
        if len(fi):
            frank[fi] = _group_rank(b_s[fi] * 128 + p_s[fi])
            for b in range(NBLK):
                mb = b_s[fi] == b
                F[k, b] = int(frank[fi][mb].max()) + 1 if mb.any() else 0
        staged.append(dict(b=b_s, p=p_s, c=c_s, nz=nz_s, st=st_s, fat=fat,
                           fol=fi, frank=frank))

    Fs = F.max(axis=0)
    foff = np.concatenate([[0], np.cumsum(Fs)]).astype(np.int64)
    fmoff = np.concatenate([[0], np.cumsum(Fs * SB)]).astype(np.int64)
    FT = max(int(foff[-1]), 1)
    FSB = max(int(fmoff[-1]), 1)
    NFX = max(int(Fs.sum()), 1)

    meta = dict(
        S1=S1s, S2=S2s, SB=SB, Fs=Fs, colb=colb, o1=o1, o2=o2,
        sidx_off=sidx_off, foff=foff, fmoff=fmoff,
        SBT=SBT, T1=T1, T2=T2, FT=FT, FSB=FSB, NFX=NFX,
    )

    bp_index = (
        np.repeat(np.arange(NBLK), RPB) * 128 + np.tile(np.arange(RPB), NBLK)
    )

    per_core = []
    for k in range(NCORES):
        s = staged[k]
        b_s, p_s, c_s, nz_s, st_s, fat = (
            s["b"], s["p"], s["c"], s["nz"], s["st"], s["fat"],
        )
        fi, frank = s["fol"], s["frank"]
        is_fol = np.zeros(len(b_s), bool)
        is_fol[fi] = True
        order = orders[k]

        egt1 = np.zeros((64, T1), BF16)
        egt2 = np.zeros((64, T2), BF16)
        noisef = np.full((128, SBT), 0.5, np.float32)
        sidx = np.full((128, NCHUNK * SBT), -1, np.int16)
        fmask = np.zeros((128, FSB), BF16)
        fxadj = np.zeros((128, FT), np.float32)
        fxcm = np.zeros((128, FT), np.float32)
        fxsi = np.zeros((128, 8 * NFX), np.int16)

        st1 = st_s == 0
        col1 = o1[b_s] + p_s * S1s[b_s] + fat
        col2 = o2[b_s] + p_s * S2s[b_s] + (fat - S1s[b_s])
        egt1[:, col1[st1]] = embed_bf[c_s[st1]].T
        egt2[:, col2[~st1]] = embed_bf[c_s[~st1]].T
        noisef[p_s, colb[b_s] + fat] = nz_s
        keep = ~is_fol
        j = c_s // CHW
        sidx[
            p_s[keep],
            sidx_off[b_s[keep]] + j[keep] * SB[b_s[keep]] + fat[keep],
        ] = (c_s[keep] - j[keep] * CHW).astype(np.int16)

        # fixups
        if len(fi):
            fb, fp, fc, fr = b_s[fi], p_s[fi], c_s[fi], frank[fi]
            fmask[fp, fmoff[fb] + fr * SB[fb] + fat[fi]] = 1
            gr = order[fb * RPB + fp] + k * RPC
            fxadj[fp, foff[fb] + fr] = adj[gr, fc]
            fxcm[fp, foff[fb] + fr] = (fc % SEGW).astype(np.float32)
        fx_flat = np.tile(
            np.arange(128, dtype=np.int64) * NSEG + (NSEG - 1), int(Fs.sum())
        ).reshape(int(Fs.sum()), 128) if Fs.sum() else np.zeros((0, 128), np.int64)
        if len(fi):
            grp_of = foff[fb] + fr  # global rank index
            fx_flat[grp_of, fp] = fp * NSEG + fc // SEGW
        if Fs.sum():
            w = np.tile(
                np.ascontiguousarray(fx_flat.reshape(-1, 16).T), (8, 1)
            ).astype(np.int16)
            fxsi[:, : w.shape[1]] = w

        adjp = np.zeros((NBLK * 128, COLS), BF16)
        embp = np.zeros((NBLK * 128, 64), np.float32)
        rows_g = order + k * RPC
        adjp[bp_index, :N] = adj[rows_g].astype(BF16)
        embp[bp_index] = embed[rows_g]

        per_core.append(
            dict(
                egt1=egt1, egt2=egt2, noisef=noisef, sidx=sidx,
                fmask=fmask, fxadj=fxadj, fxcm=fxcm, fxsi=fxsi,
                adjp=adjp, embp=embp,
            )
        )
    return per_core, orders, meta


def _emulate_core(m, meta, W1, b1, W2, b2):
    """Numpy emulation of the device program for one core (testing aid)."""
    S1, S2, SB = meta["S1"], meta["S2"], meta["SB"]
    Fs, colb = meta["Fs"], meta["colb"]
    o1, o2, sidx_off = meta["o1"], meta["o2"], meta["sidx_off"]
    foff, fmoff = meta["foff"], meta["fmoff"]

    W1a = W1[0:64].astype(np.float32)
    W1b = W1[64:128].astype(np.float32)
    w2 = W2.reshape(-1).astype(BF16).astype(np.float32)
    W1ab = W1a.astype(BF16).astype(np.float32)
    W1bb = W1b.astype(BF16).astype(np.float32)
    crow = m["_crow"]  # [64] f32, passed in by caller

    out = np.zeros((NBLK * 128, COLS), np.float32)
    egt1 = m["egt1"].astype(np.float32)
    egt2 = m["egt2"].astype(np.float32)
    embp = m["embp"]

    sfat = np.zeros((128, meta["SBT"]), np.float32)
    for b in range(NBLK):
        Eblk = embp[b * 128 : (b + 1) * 128]
        PAT = (Eblk @ W1a + crow).T
        PBT = (Eblk @ W1b + crow).T
        for st, (S_b, oo, egt, Wo, PT) in enumerate(
            [(S1[b], o1[b], egt1, W1bb, PAT), (S2[b], o2[b], egt2, W1ab, PBT)]
        ):
            L = 128 * S_b
            pre = Wo.T @ egt[:, oo : oo + L] + np.repeat(PT, S_b, axis=1)
            pre = np.maximum(pre.astype(BF16).astype(np.float32), 0.0)
            sarr = w2 @ pre
            c0 = colb[b] + (0 if st == 0 else S1[b])
            sfat[:, c0 : c0 + S_b] = sarr.reshape(128, S_b)

    nz = m["noisef"]
    z = np.log(nz) - np.log1p(-nz) + sfat + float(b2)
    gate = 1.0 / (1.0 + np.exp(-z))
    gatebf = (gate * 0.5).astype(BF16)

    for b in range(NBLK):
        gsl = gatebf[:, colb[b] : colb[b] + SB[b]]
        for j in range(NCHUNK):
            idx = m["sidx"][
                :, sidx_off[b] + j * SB[b] : sidx_off[b] + (j + 1) * SB[b]
            ]
            mask = np.zeros((128, CHW), BF16)
            rows, cols_ = np.where(idx >= 0)
            mask[rows, idx[rows, cols_]] = gsl[rows, cols_]
            prod = (m["adjp"][b * 128 : (b + 1) * 128, j * CHW : (j + 1) * CHW]
                    * mask).astype(BF16)
            out[b * 128 : (b + 1) * 128, j * CHW : (j + 1) * CHW] = prod
        for r in range(Fs[b]):
            fm = m["fmask"][:, fmoff[b] + r * SB[b] : fmoff[b] + (r + 1) * SB[b]]
            fxg = (gsl.astype(np.float32) * fm.astype(np.float32)).sum(axis=1)
            t0 = fxg * m["fxadj"][:, foff[b] + r]
            cm = m["fxcm"][:, foff[b] + r].astype(np.int64)
            grp = int(foff[b]) + r
            for p in range(128):
                si = int(m["fxsi"][p % 16, grp * 8 + p // 16])
                seg = si - p * NSEG
                if seg != NSEG - 1:
                    colx = seg * SEGW + int(cm[p])
                    out[b * 128 + p, colx] += np.float32(BF16(t0[p]))
    return out


def _build_program(meta, b2f):
    import concourse.bacc as bacc
    import concourse.mybir as mybir
    import concourse.tile as tile
    from concourse.masks import make_identity

    f32 = mybir.dt.float32
    bf16 = mybir.dt.bfloat16
    i16 = mybir.dt.int16
    add = mybir.AluOpType.add
    mult = mybir.AluOpType.mult
    subtract = mybir.AluOpType.subtract
    is_equal = mybir.AluOpType.is_equal
    AF = mybir.ActivationFunctionType

    S1, S2, SB = meta["S1"], meta["S2"], meta["SB"]
    Fs, colb = meta["Fs"], meta["colb"]
    o1, o2, sidx_off = meta["o1"], meta["o2"], meta["sidx_off"]
    foff, fmoff = meta["foff"], meta["fmoff"]
    SBT, T1, T2, FT, FSB, NFX = (
        meta["SBT"], meta["T1"], meta["T2"], meta["FT"], meta["FSB"],
        meta["NFX"],
    )
    have_fx = int(Fs.sum()) > 0
    SMAX = 128 * int(max(S1.max(), S2.max()))

    nc = bacc.Bacc()

    egt1p = nc.declare_dram_parameter("egt1", [64, T1], bf16, isOutput=False)
    egt2p = nc.declare_dram_parameter("egt2", [64, T2], bf16, isOutput=False)
    noisep = nc.declare_dram_parameter("noisef", [128, SBT], f32, isOutput=False)
    sidxp = nc.declare_dram_parameter("sidx", [128, NCHUNK * SBT], i16, isOutput=False)
    fmaskp = nc.declare_dram_parameter("fmask", [128, FSB], bf16, isOutput=False)
    fxadjp = nc.declare_dram_parameter("fxadj", [128, FT], f32, isOutput=False)
    fxcmp = nc.declare_dram_parameter("fxcm", [128, FT], f32, isOutput=False)
    fxsip = nc.declare_dram_parameter("fxsi", [128, 8 * NFX], i16, isOutput=False)
    adjp = nc.declare_dram_parameter("adjp", [NBLK * 128, COLS], bf16, isOutput=False)
    embp = nc.declare_dram_parameter("embp", [NBLK * 128, 64], f32, isOutput=False)
    e5p = nc.declare_dram_parameter("e5", [64, 1], f32, isOutput=False)
    w1afp = nc.declare_dram_parameter("w1af", [64, 64], f32, isOutput=False)
    w1bfp = nc.declare_dram_parameter("w1bf", [64, 64], f32, isOutput=False)
    w1cfp = nc.declare_dram_parameter("w1cf", [64, 64], f32, isOutput=False)
    w1abp = nc.declare_dram_parameter("w1ab", [64, 64], bf16, isOutput=False)
    w1bbp = nc.declare_dram_parameter("w1bb", [64, 64], bf16, isOutput=False)
    w2bp = nc.declare_dram_parameter("w2b", [64, 1], bf16, isOutput=False)
    b1rp = nc.declare_dram_parameter("b1r", [1, 64], f32, isOutput=False)
    iotp = nc.declare_dram_parameter("iot", [128, 128], f32, isOutput=False)
    outp = nc.declare_dram_parameter("out", [NBLK * 128, COLS], bf16, isOutput=True)

    sdram = nc.dram_tensor("sdram", [2 * NBLK, SMAX], f32)

    with tile.TileContext(nc) as tc:
        with (
            tc.tile_pool(name="const", bufs=1) as cp,
            tc.tile_pool(name="blk", bufs=2) as bp,
            tc.tile_pool(name="srowp", bufs=1) as srp,
            tc.tile_pool(name="work", bufs=3) as wp,
            tc.tile_pool(name="small", bufs=2) as sp,
            tc.tile_pool(name="psA", bufs=3, space="PSUM") as ppA,
            tc.tile_pool(name="psB", bufs=2, space="PSUM") as ppB,
            tc.tile_pool(name="psC", bufs=1, space="PSUM") as ppC,
        ):
            identity = cp.tile([128, 128], f32)
            make_identity(nc, identity[:])
            w1af = cp.tile([64, 64], f32)
            nc.sync.dma_start(out=w1af[:], in_=w1afp[:, :])
            w1bf = cp.tile([64, 64], f32)
            nc.sync.dma_start(out=w1bf[:], in_=w1bfp[:, :])
            w1cf = cp.tile([64, 64], f32)
            nc.sync.dma_start(out=w1cf[:], in_=w1cfp[:, :])
            w1ab = cp.tile([64, 64], bf16)
            nc.sync.dma_start(out=w1ab[:], in_=w1abp[:, :])
            w1bb = cp.tile([64, 64], bf16)
            nc.sync.dma_start(out=w1bb[:], in_=w1bbp[:, :])
            w2b = cp.tile([64, 1], bf16)
            nc.sync.dma_start(out=w2b[:], in_=w2bp[:, :])
            b1t = cp.tile([1, 64], f32)
            nc.sync.dma_start(out=b1t[:], in_=b1rp[:, :])
            e5t = cp.tile([64, 1], f32)
            nc.sync.dma_start(out=e5t[:], in_=e5p[:, :])
            iot = cp.tile([128, 128], f32)
            nc.sync.dma_start(out=iot[:], in_=iotp[:, :])
            ones128 = cp.tile([1, 128], f32)
            nc.vector.memset(ones128[:], 1.0)
            noiset = cp.tile([128, SBT], f32)
            nc.sync.dma_start(out=noiset[:], in_=noisep[:, :])
            sidxt = cp.tile([128, NCHUNK * SBT], i16)
            nc.sync.dma_start(out=sidxt[:], in_=sidxp[:, :])
            if have_fx:
                fmaskt = cp.tile([128, FSB], bf16)
                nc.sync.dma_start(out=fmaskt[:], in_=fmaskp[:, :])
                fxadjt = cp.tile([128, FT], f32)
                nc.sync.dma_start(out=fxadjt[:], in_=fxadjp[:, :])
                fxcmt = cp.tile([128, FT], f32)
                nc.sync.dma_start(out=fxcmt[:], in_=fxcmp[:, :])
                fxsit = cp.tile([128, 8 * NFX], i16)
                nc.sync.dma_start(out=fxsit[:], in_=fxsip[:, :])
            sfat = cp.tile([128, SBT], f32)
            gatebf = cp.tile([128, SBT], bf16)

            cps = ppC.tile([1, 64], f32, tag="cps")
            nc.tensor.matmul(cps[:], lhsT=e5t[:], rhs=w1cf[:], start=True, stop=True)
            crow = cp.tile([1, 64], f32)
            nc.vector.tensor_tensor(out=crow[:], in0=cps[:], in1=b1t[:], op=add)

            for b in range(NBLK):
                # dest-side tables PAT/PBT [64, 128] = (Eblk @ W1x + c)^T
                et = bp.tile([128, 64], f32, tag="et")
                nc.sync.dma_start(out=et[:], in_=embp[b * 128 : (b + 1) * 128, :])
                tps = ppC.tile([64, 128], f32, tag="blkps")
                nc.tensor.transpose(tps[:, :], et[:, :], identity[:, :])
                ebT = bp.tile([64, 128], f32, tag="ebT")
                nc.scalar.copy(out=ebT[:], in_=tps[:])
                patp = ppC.tile([64, 128], f32, tag="blkps")
                nc.tensor.matmul(patp[:], lhsT=w1af[:], rhs=ebT[:], start=True, stop=False)
                nc.tensor.matmul(patp[:], lhsT=crow[:], rhs=ones128[:], start=False, stop=True)
                patf = bp.tile([64, 128], f32, tag="patf")
                nc.scalar.copy(out=patf[:], in_=patp[:])
                pbtp = ppC.tile([64, 128], f32, tag="blkps")
                nc.tensor.matmul(pbtp[:], lhsT=w1bf[:], rhs=ebT[:], start=True, stop=False)
                nc.tensor.matmul(pbtp[:], lhsT=crow[:], rhs=ones128[:], start=False, stop=True)
                pbtf = bp.tile([64, 128], f32, tag="pbtf")
                nc.scalar.copy(out=pbtf[:], in_=pbtp[:])

                ci = 0
                for st in (0, 1):
                    S_b = int(S1[b] if st == 0 else S2[b])
                    oo = int(o1[b] if st == 0 else o2[b])
                    egtp = egt1p if st == 0 else egt2p
                    wmat = w1bb if st == 0 else w1ab
                    pt = patf if st == 0 else pbtf
                    L = 128 * S_b
                    g = max(min(512 // S_b, 128), 1)

                    egt = bp.tile([64, L], bf16, tag="egt")
                    nc.sync.dma_start(out=egt[:], in_=egtp[:, oo : oo + L])
                    pre = bp.tile([64, L], bf16, tag="pre")
                    p0 = 0
                    while p0 < 128:
                        gg = min(g, 128 - p0)
                        Lc = gg * S_b
                        c0 = p0 * S_b
                        pps = ppA.tile([64, Lc], f32, tag="pps")
                        nc.tensor.matmul(
                            pps[:], lhsT=wmat[:], rhs=egt[:, c0 : c0 + Lc],
                            start=True, stop=True,
                        )
                        pt_b = (
                            pt[:, p0 : p0 + gg]
                            .rearrange("h (g o) -> h g o", o=1)
                            .to_broadcast([64, gg, S_b])
                        )
                        nc.vector.tensor_tensor(
                            out=pre[:, c0 : c0 + Lc].rearrange(
                                "h (g s) -> h g s", s=S_b
                            ),
                            in0=pps[:].rearrange("h (g s) -> h g s", s=S_b),
                            in1=pt_b,
                            op=add,
                        )
                        p0 += gg
                    nc.scalar.activation(out=pre[:], in_=pre[:], func=AF.Relu)
                    srow = srp.tile([1, L], f32, tag="srow")
                    p0 = 0
                    while p0 < 128:
                        gg = min(g, 128 - p0)
                        Lc = gg * S_b
                        c0 = p0 * S_b
                        sps = ppB.tile([1, Lc], f32, tag="sps")
                        nc.tensor.matmul(
                            sps[:], lhsT=w2b[:], rhs=pre[:, c0 : c0 + Lc],
                            start=True, stop=True,
                        )
                        if ci % 2 == 0:
                            nc.scalar.copy(out=srow[:, c0 : c0 + Lc], in_=sps[:])
                        else:
                            nc.vector.tensor_scalar_add(
                                srow[:, c0 : c0 + Lc], sps[:], 0.0
                            )
                        ci += 1
                        p0 += gg
                    # DRAM round-trip reshape [1, L] -> [128, S_b]
                    slot = 2 * b + st
                    nc.sync.dma_start(out=sdram[slot : slot + 1, 0:L], in_=srow[:])
                    cdst = int(colb[b]) + (0 if st == 0 else int(S1[b]))
                    nc.sync.dma_start(
                        out=sfat[:, cdst : cdst + S_b],
                        in_=sdram[slot : slot + 1, 0:L].rearrange(
                            "o (p s) -> (o p) s", p=128
                        ),
                    )

                # gate math on fat slice [128, SB_b]
                sb = int(SB[b])
                c0 = int(colb[b])
                nzs = noiset[:, c0 : c0 + sb]
                om = sp.tile([128, sb], f32, tag="om")
                nc.vector.tensor_scalar(
                    out=om[:], in0=nzs, scalar1=-1.0, scalar2=1.0,
                    op0=mult, op1=add,
                )
                ln1 = sp.tile([128, sb], f32, tag="ln1")
                nc.scalar.activation(out=ln1[:], in_=nzs, func=AF.Ln)
                ln2 = sp.tile([128, sb], f32, tag="ln2")
                nc.scalar.activation(out=ln2[:], in_=om[:], func=AF.Ln)
                z = sp.tile([128, sb], f32, tag="z")
                nc.vector.scalar_tensor_tensor(
                    out=z[:], in0=ln1[:], scalar=b2f, in1=ln2[:],
                    op0=add, op1=subtract,
                )
                nc.vector.tensor_tensor(
                    out=z[:], in0=z[:], in1=sfat[:, c0 : c0 + sb], op=add
                )
                gf = sp.tile([128, sb], f32, tag="gf")
                nc.scalar.activation(out=gf[:], in_=z[:], func=AF.Sigmoid)
                nc.vector.tensor_scalar_mul(gatebf[:, c0 : c0 + sb], gf[:], 0.5)

                # dense mask chunks + multiply + writeout
                for j in range(NCHUNK):
                    adjt = wp.tile([128, CHW], bf16, tag="adjt")
                    nc.sync.dma_start(
                        out=adjt[:],
                        in_=adjp[b * 128 : (b + 1) * 128, j * CHW : (j + 1) * CHW],
                    )
                    mask = wp.tile([128, CHW], bf16, tag="mask")
                    nc.gpsimd.local_scatter(
                        out_ap=mask[:],
                        data_ap=gatebf[:, c0 : c0 + sb],
                        idxs_ap=sidxt[
                            :,
                            int(sidx_off[b]) + j * sb : int(sidx_off[b])
                            + (j + 1) * sb,
                        ],
                        channels=128,
                        num_elems=CHW,
                        num_idxs=sb,
                    )
                    nc.vector.tensor_tensor(
                        out=mask[:], in0=mask[:], in1=adjt[:], op=mult
                    )
                    nc.sync.dma_start(
                        out=outp[b * 128 : (b + 1) * 128, j * CHW : (j + 1) * CHW],
                        in_=mask[:],
                    )

                # fixup scatter-adds (rare duplicate cells)
                for r in range(int(Fs[b])):
                    fm = fmaskt[
                        :, int(fmoff[b]) + r * sb : int(fmoff[b]) + (r + 1) * sb
                    ]
                    prod = sp.tile([128, sb], bf16, tag="fprod")
                    nc.vector.tensor_tensor(
                        out=prod[:], in0=gatebf[:, c0 : c0 + sb], in1=fm, op=mult
                    )
                    fxg = sp.tile([128, 1], f32, tag="fxg")
                    nc.vector.tensor_reduce(
                        out=fxg[:], in_=prod[:], axis=mybir.AxisListType.X, op=add
                    )
                    t0 = sp.tile([128, 1], f32, tag="t0")
                    nc.vector.tensor_tensor(
                        out=t0[:], in0=fxg[:],
                        in1=fxadjt[:, int(foff[b]) + r : int(foff[b]) + r + 1],
                        op=mult,
                    )
                    oh = sp.tile([128, 128], f32, tag="oh")
                    nc.vector.tensor_tensor(
                        out=oh[:], in0=iot[:],
                        in1=fxcmt[
                            :, int(foff[b]) + r : int(foff[b]) + r + 1
                        ].to_broadcast([128, 128]),
                        op=is_equal,
                    )
                    pay = sp.tile([128, 128], bf16, tag="pay")
                    nc.vector.tensor_tensor(
                        out=pay[:], in0=oh[:], in1=t0[:].to_broadcast([128, 128]),
                        op=mult,
                    )
                    grp = int(foff[b]) + r
                    out_view = outp[b * 128 : (b + 1) * 128, :].rearrange(
                        "p (s w) -> (p s) w", w=SEGW
                    )
                    nc.gpsimd.dma_scatter_add(
                        out_ap=out_view,
                        in_ap=pay[:].rearrange("p (s d) -> p s d", d=SEGW),
                        idxs_ap=fxsit[:, grp * 8 : (grp + 1) * 8],
                        num_idxs=128,
                        num_idxs_reg=128,
                        elem_size=SEGW,
                    )

    nc.compile()
    return nc


def kernel(embed, row, col, adj, noise, W1, b1, W2, b2, node_idx):
    from concourse.bass_utils import run_bass_kernel_spmd

    embed = np.ascontiguousarray(np.asarray(embed), dtype=np.float32)
    adj = np.ascontiguousarray(np.asarray(adj), dtype=np.float32)
    W1 = np.ascontiguousarray(np.asarray(W1), dtype=np.float32)
    b1 = np.ascontiguousarray(np.asarray(b1), dtype=np.float32).ravel()
    W2 = np.ascontiguousarray(np.asarray(W2), dtype=np.float32)
    b2f = float(np.asarray(b2, dtype=np.float32).ravel()[0])
    nidx = int(np.asarray(node_idx))

    per_core, orders, meta = _prep_host(row, col, noise, adj, embed)
    nc = _build_program(meta, b2f)

    w1a = np.ascontiguousarray(W1[0:64])
    w1b = np.ascontiguousarray(W1[64:128])
    w1c = np.ascontiguousarray(W1[128:192])
    common = dict(
        e5=np.ascontiguousarray(embed[nidx].reshape(64, 1)),
        w1af=w1a, w1bf=w1b, w1cf=w1c,
        w1ab=np.ascontiguousarray(w1a.astype(BF16)),
        w1bb=np.ascontiguousarray(w1b.astype(BF16)),
        w2b=np.ascontiguousarray(W2.reshape(64, 1).astype(BF16)),
        b1r=np.ascontiguousarray(b1.reshape(1, 64)),
        iot=np.ascontiguousarray(
            np.tile(np.arange(128, dtype=np.float32), (128, 1))
        ),
    )
    in_maps = []
    for k in range(NCORES):
        mcore = dict(per_core[k])
        mcore.update(common)
        in_maps.append(mcore)

    res = run_bass_kernel_spmd(nc, in_maps, list(range(NCORES)))
    kernel.last_exec_time_ns = res.exec_time_ns
    it = getattr(res, "instructions_and_trace", None)
    kernel.last_trace_path = it[1] if it else None

    bp_index = (
        np.repeat(np.arange(NBLK), RPB) * 128 + np.tile(np.arange(RPB), NBLK)
    )
    out = np.empty((N, N), np.float32)
    for k in range(NCORES):
        o = np.asarray(res.results[k]["out"])[:, :N].astype(np.float32)
        out[orders[k] + k * RPC] = o[bp_index]
    return out


kernel.last_exec_time_ns = None
kernel.last_trace_path = None


# revision 6
# speedup vs baseline: 6.3952x; 1.0941x over previous
"""Trainium2 Bass kernel for the GNN ExplainModule (masked adjacency).

Dense row-block design (8 NeuronCores, row-sharded output):
  - Core k owns rows [k*1250, (k+1)*1250). Rows are re-ordered by token
    count and grouped into 10 blocks of 125 rows (partitions 0-124).
  - Every mask contribution ("token") for cell (r, c) lives in the
    partition of its dest row r. Two streams per block share one slot
    grid of width S_b: stream1 = copy1 tokens (dest side uses W1a) on
    hidden partitions 0-63, stream2 = copy2 (dest side uses W1b) on
    partitions 64-127.
  - MLP runs in transposed layout [hidden x tokens]: host pre-gathers
    embed[c] columns (bf16) per token; PE computes (E @ W1x)^T per
    <=512-token chunk into a [128, L] pre tile (both streams stacked);
    the dest-row term (Eblk @ W1y + c)^T is added via a partition-run
    broadcast view; one relu covers both streams; a single PE matvec
    with the block-diagonal [w2;0|0;w2] weight yields both streams'
    logits [2, L]; a DRAM round-trip reshapes them into the fat
    [128, S] layout where the concrete gate is computed.
  - gpsimd local_scatter turns each block's gates into a dense
    [128, 2000] mask chunk; DVE multiplies by the adj chunk (bf16); the
    product is written out densely. Duplicate cells (same (r,c) fed by
    several edges) keep the first token in the dense path; the rare
    followers (~300/core) are applied afterwards with per-rank
    dma_scatter_add CCE adds of one-hot payloads.
"""

import sys

import numpy as np

for _p in ("/opt/trn_rl_repo",):
    if _p not in sys.path:
        sys.path.insert(0, _p)

import ml_dtypes

BF16 = ml_dtypes.bfloat16

N = 10000
D = 64
NCORES = 8
RPC = N // NCORES  # 1250 rows per core
NBLK = 10
RPB = RPC // NBLK  # 125 real rows per block
COLS = 10240  # padded row pitch (80 segs of 128)
NCHUNK = 5
CHW = 2000  # dense chunk width
SEGW = 128  # scatter-add segment width (bf16 -> 256B)
NSEG = COLS // SEGW  # 80


def _group_rank(key):
    """Rank of each element within its key-group (appearance order)."""
    o = np.argsort(key, kind="stable")
    ks = key[o]
    starts = np.flatnonzero(np.concatenate([[True], ks[1:] != ks[:-1]]))
    sizes = np.diff(np.concatenate([starts, [len(ks)]]))
    rank_sorted = np.arange(len(ks)) - np.repeat(starts, sizes)
    rank = np.empty(len(key), np.int64)
    rank[o] = rank_sorted
    return rank


def _prep_host(row, col, noise, adj, embed):
    """Route tokens, balance rows into blocks, build all per-core arrays."""
    row = np.asarray(row).astype(np.int64).ravel()
    col = np.asarray(col).astype(np.int64).ravel()
    noise = np.asarray(noise).astype(np.float32).ravel()
    adj = np.asarray(adj, dtype=np.float32)
    embed = np.asarray(embed, dtype=np.float32)
    embed_bf = embed.astype(BF16)

    E = row.shape[0]
    t_r = np.concatenate([row, col])  # dest row
    t_c = np.concatenate([col, row])  # dest col == other endpoint
    t_nz = np.concatenate([noise, noise])
    t_st = np.concatenate([np.zeros(E, np.int8), np.ones(E, np.int8)])
    core_of = t_r // RPC

    # ---- pass 1: per-core row stats and block structure ----
    per_core_tok = []
    orders = []
    Sm = np.zeros((NCORES, NBLK), np.int64)  # shared slot width per block
    for k in range(NCORES):
        m = core_of == k
        r_loc = (t_r[m] - k * RPC).astype(np.int64)
        cc = t_c[m].astype(np.int64)
        nz = t_nz[m]
        st = t_st[m].astype(np.int64)
        n1 = np.bincount(r_loc[st == 0], minlength=RPC)
        n2 = np.bincount(r_loc[st == 1], minlength=RPC)
        nm = np.maximum(n1, n2)
        order = np.argsort(-nm, kind="stable")
        orders.append(order)
        blk_of_row = np.empty(RPC, np.int64)
        part_of_row = np.empty(RPC, np.int64)
        for b in range(NBLK):
            rows_b = order[b * RPB : (b + 1) * RPB]
            blk_of_row[rows_b] = b
            part_of_row[rows_b] = np.arange(RPB)
            Sm[k, b] = max(int(nm[rows_b].max()), 1)
        per_core_tok.append((r_loc, cc, nz, st, blk_of_row, part_of_row))

    # SPMD-static shapes: max over cores
    Ss = Sm.max(axis=0)  # shared per-stream slot count per block
    SB = 2 * Ss  # fat width per block (even by construction)

    colb = np.concatenate([[0], np.cumsum(SB)]).astype(np.int64)
    SBT = int(colb[-1])
    oT = np.concatenate([[0], np.cumsum(128 * Ss)]).astype(np.int64)
    TT = int(oT[-1])  # per-stream token columns
    sidx_off = np.concatenate([[0], np.cumsum(NCHUNK * SB)]).astype(np.int64)

    # ---- pass 2: slots, duplicates, fixup ranks ----
    staged = []
    F = np.zeros((NCORES, NBLK), np.int64)
    for k in range(NCORES):
        r_loc, cc, nz, st, blk_of_row, part_of_row = per_core_tok[k]
        b_s = blk_of_row[r_loc]
        p_s = part_of_row[r_loc]
        key = ((b_s * 128 + p_s) * 2 + st) * N + cc
        o = np.argsort(key, kind="stable")
        b_s, p_s, c_s, nz_s, st_s = b_s[o], p_s[o], cc[o], nz[o], st[o]
        slot = _group_rank((b_s * 128 + p_s) * 2 + st_s)
        fat = np.where(st_s == 0, slot, Ss[b_s] + slot)
        crank = _group_rank((b_s * 128 + p_s) * N + c_s)
        is_fol = crank > 0
        frank = np.full(len(b_s), -1, np.int64)
        fi = np.flatnonzero(is_fol)
        if len(fi):
            frank[fi] = _group_rank(b_s[fi] * 128 + p_s[fi])
            for b in range(NBLK):
                mb = b_s[fi] == b
                F[k, b] = int(frank[fi][mb].max()) + 1 if mb.any() else 0
        staged.append(dict(b=b_s, p=p_s, c=c_s, nz=nz_s, st=st_s, fat=fat,
                           fol=fi, frank=frank))

    Fs = F.max(axis=0)
    foff = np.concatenate([[0], np.cumsum(Fs)]).astype(np.int64)
    fmoff = np.concatenate([[0], np.cumsum(Fs * SB)]).astype(np.int64)
    FT = max(int(foff[-1]), 1)
    FSB = max(int(fmoff[-1]), 1)
    NFX = max(int(Fs.sum()), 1)

    meta = dict(
        Ss=Ss, SB=SB, Fs=Fs, colb=colb, oT=oT,
        sidx_off=sidx_off, foff=foff, fmoff=fmoff,
        SBT=SBT, TT=TT, FT=FT, FSB=FSB, NFX=NFX,
    )

    bp_index = (
        np.repeat(np.arange(NBLK), RPB) * 128 + np.tile(np.arange(RPB), NBLK)
    )

    per_core = []
    for k in range(NCORES):
        s = staged[k]
        b_s, p_s, c_s, nz_s, st_s, fat = (
            s["b"], s["p"], s["c"], s["nz"], s["st"], s["fat"],
        )
        fi, frank = s["fol"], s["frank"]
        is_fol = np.zeros(len(b_s), bool)
        is_fol[fi] = True
        order = orders[k]

        egt1 = np.zeros((64, TT), BF16)
        egt2 = np.zeros((64, TT), BF16)
        noisef = np.full((128, SBT), 0.5, np.float32)
        sidx = np.full((128, NCHUNK * SBT), -1, np.int16)
        fmask = np.zeros((128, FSB), BF16)
        fxadj = np.zeros((128, FT), np.float32)
        fxcm = np.zeros((128, FT), np.float32)
        fxsi = np.zeros((128, 8 * NFX), np.int16)

        st1 = st_s == 0
        colx = oT[b_s] + p_s * Ss[b_s] + np.where(st1, fat, fat - Ss[b_s])
        egt1[:, colx[st1]] = embed_bf[c_s[st1]].T
        egt2[:, colx[~st1]] = embed_bf[c_s[~st1]].T
        noisef[p_s, colb[b_s] + fat] = nz_s
        keep = ~is_fol
        j = c_s // CHW
        sidx[
            p_s[keep],
            sidx_off[b_s[keep]] + j[keep] * SB[b_s[keep]] + fat[keep],
        ] = (c_s[keep] - j[keep] * CHW).astype(np.int16)

        if len(fi):
            fb, fp, fc, fr = b_s[fi], p_s[fi], c_s[fi], frank[fi]
            fmask[fp, fmoff[fb] + fr * SB[fb] + fat[fi]] = 1
            gr = order[fb * RPB + fp] + k * RPC
            fxadj[fp, foff[fb] + fr] = adj[gr, fc]
            fxcm[fp, foff[fb] + fr] = (fc % SEGW).astype(np.float32)
        nfx_tot = int(Fs.sum())
        fx_flat = (
            np.tile(np.arange(128, dtype=np.int64) * NSEG + (NSEG - 1), nfx_tot)
            .reshape(nfx_tot, 128)
            if nfx_tot
            else np.zeros((0, 128), np.int64)
        )
        if len(fi):
            fx_flat[foff[fb] + fr, fp] = fp * NSEG + fc // SEGW
        if nfx_tot:
            w = np.tile(
                np.ascontiguousarray(fx_flat.reshape(-1, 16).T), (8, 1)
            ).astype(np.int16)
            fxsi[:, : w.shape[1]] = w

        adjp = np.zeros((NBLK * 128, COLS), BF16)
        embp = np.zeros((NBLK * 128, 64), np.float32)
        rows_g = order + k * RPC
        adjp[bp_index, :N] = adj[rows_g].astype(BF16)
        embp[bp_index] = embed[rows_g]

        per_core.append(
            dict(
                egt1=egt1, egt2=egt2, noisef=noisef, sidx=sidx,
                fmask=fmask, fxadj=fxadj, fxcm=fxcm, fxsi=fxsi,
                adjp=adjp, embp=embp,
            )
        )
    return per_core, orders, meta


def _emulate_core(m, meta, W1, b1, W2, b2):
    """Numpy emulation of the device program for one core (testing aid)."""
    Ss, SB = meta["Ss"], meta["SB"]
    Fs, colb, oT = meta["Fs"], meta["colb"], meta["oT"]
    sidx_off, foff, fmoff = meta["sidx_off"], meta["foff"], meta["fmoff"]

    W1a = W1[0:64].astype(np.float32)
    W1b = W1[64:128].astype(np.float32)
    w2 = W2.reshape(-1).astype(BF16).astype(np.float32)
    W1ab = W1a.astype(BF16).astype(np.float32)
    W1bb = W1b.astype(BF16).astype(np.float32)
    crow = m["_crow"]

    out = np.zeros((NBLK * 128, COLS), np.float32)
    egt1 = m["egt1"].astype(np.float32)
    egt2 = m["egt2"].astype(np.float32)
    embp = m["embp"]

    sfat = np.zeros((128, meta["SBT"]), np.float32)
    for b in range(NBLK):
        Eblk = embp[b * 128 : (b + 1) * 128]
        PAT = (Eblk @ W1a + crow).T
        PBT = (Eblk @ W1b + crow).T
        S_b = Ss[b]
        L = 128 * S_b
        pre1 = W1bb.T @ egt1[:, oT[b] : oT[b] + L] + np.repeat(PAT, S_b, axis=1)
        pre2 = W1ab.T @ egt2[:, oT[b] : oT[b] + L] + np.repeat(PBT, S_b, axis=1)
        pre1 = np.maximum(pre1.astype(BF16).astype(np.float32), 0.0)
        pre2 = np.maximum(pre2.astype(BF16).astype(np.float32), 0.0)
        s1 = (w2 @ pre1).reshape(128, S_b)
        s2 = (w2 @ pre2).reshape(128, S_b)
        sfat[:, colb[b] : colb[b] + S_b] = s1
        sfat[:, colb[b] + S_b : colb[b] + SB[b]] = s2

    nz = m["noisef"]
    z = np.log(nz) - np.log1p(-nz) + sfat + float(b2)
    gate = 1.0 / (1.0 + np.exp(-z))
    gatebf = (gate * 0.5).astype(BF16)

    for b in range(NBLK):
        gsl = gatebf[:, colb[b] : colb[b] + SB[b]]
        for j in range(NCHUNK):
            idx = m["sidx"][
                :, sidx_off[b] + j * SB[b] : sidx_off[b] + (j + 1) * SB[b]
            ]
            mask = np.zeros((128, CHW), BF16)
            rows, cols_ = np.where(idx >= 0)
            mask[rows, idx[rows, cols_]] = gsl[rows, cols_]
            prod = (
                m["adjp"][b * 128 : (b + 1) * 128, j * CHW : (j + 1) * CHW]
                * mask
            ).astype(BF16)
            out[b * 128 : (b + 1) * 128, j * CHW : (j + 1) * CHW] = prod
        for r in range(Fs[b]):
            fm = m["fmask"][:, fmoff[b] + r * SB[b] : fmoff[b] + (r + 1) * SB[b]]
            fxg = (gsl.astype(np.float32) * fm.astype(np.float32)).sum(axis=1)
            t0 = fxg * m["fxadj"][:, foff[b] + r]
            cm = m["fxcm"][:, foff[b] + r].astype(np.int64)
            grp = int(foff[b]) + r
            for p in range(128):
                si = int(m["fxsi"][p % 16, grp * 8 + p // 16])
                seg = si - p * NSEG
                if seg != NSEG - 1:
                    colx = seg * SEGW + int(cm[p])
                    out[b * 128 + p, colx] += np.float32(BF16(t0[p]))
    return out


def _build_program(meta, b2f):
    import concourse.bacc as bacc
    import concourse.mybir as mybir
    import concourse.tile as tile
    from concourse.masks import make_identity

    f32 = mybir.dt.float32
    bf16 = mybir.dt.bfloat16
    i16 = mybir.dt.int16
    add = mybir.AluOpType.add
    mult = mybir.AluOpType.mult
    subtract = mybir.AluOpType.subtract
    is_equal = mybir.AluOpType.is_equal
    AF = mybir.ActivationFunctionType

    Ss, SB = meta["Ss"], meta["SB"]
    Fs, colb, oT = meta["Fs"], meta["colb"], meta["oT"]
    sidx_off, foff, fmoff = meta["sidx_off"], meta["foff"], meta["fmoff"]
    SBT, TT, FT, FSB, NFX = (
        meta["SBT"], meta["TT"], meta["FT"], meta["FSB"], meta["NFX"],
    )
    have_fx = int(Fs.sum()) > 0
    LMAX = 128 * int(Ss.max())

    nc = bacc.Bacc()

    egt1p = nc.declare_dram_parameter("egt1", [64, TT], bf16, isOutput=False)
    egt2p = nc.declare_dram_parameter("egt2", [64, TT], bf16, isOutput=False)
    noisep = nc.declare_dram_parameter("noisef", [128, SBT], f32, isOutput=False)
    sidxp = nc.declare_dram_parameter("sidx", [128, NCHUNK * SBT], i16, isOutput=False)
    fmaskp = nc.declare_dram_parameter("fmask", [128, FSB], bf16, isOutput=False)
    fxadjp = nc.declare_dram_parameter("fxadj", [128, FT], f32, isOutput=False)
    fxcmp = nc.declare_dram_parameter("fxcm", [128, FT], f32, isOutput=False)
    fxsip = nc.declare_dram_parameter("fxsi", [128, 8 * NFX], i16, isOutput=False)
    adjp = nc.declare_dram_parameter("adjp", [NBLK * 128, COLS], bf16, isOutput=False)
    embp = nc.declare_dram_parameter("embp", [NBLK * 128, 64], f32, isOutput=False)
    e5p = nc.declare_dram_parameter("e5", [64, 1], f32, isOutput=False)
    w1afp = nc.declare_dram_parameter("w1af", [64, 64], f32, isOutput=False)
    w1bfp = nc.declare_dram_parameter("w1bf", [64, 64], f32, isOutput=False)
    w1cfp = nc.declare_dram_parameter("w1cf", [64, 64], f32, isOutput=False)
    w1abp = nc.declare_dram_parameter("w1ab", [64, 64], bf16, isOutput=False)
    w1bbp = nc.declare_dram_parameter("w1bb", [64, 64], bf16, isOutput=False)
    w2dp = nc.declare_dram_parameter("w2d", [128, 2], bf16, isOutput=False)
    b1rp = nc.declare_dram_parameter("b1r", [1, 64], f32, isOutput=False)
    iotp = nc.declare_dram_parameter("iot", [128, 128], f32, isOutput=False)
    outp = nc.declare_dram_parameter("out", [NBLK * 128, COLS], bf16, isOutput=True)

    sdram = nc.dram_tensor("sdram", [2 * NBLK, LMAX], f32)

    with tile.TileContext(nc) as tc:
        with (
            tc.tile_pool(name="const", bufs=1) as cp,
            tc.tile_pool(name="blk", bufs=2) as bp,
            tc.tile_pool(name="srowp", bufs=1) as srp,
            tc.tile_pool(name="work", bufs=3) as wp,
            tc.tile_pool(name="small", bufs=2) as sp,
            tc.tile_pool(name="psA", bufs=4, space="PSUM") as ppA,
            tc.tile_pool(name="psB", bufs=2, space="PSUM") as ppB,
            tc.tile_pool(name="psC", bufs=1, space="PSUM") as ppC,
        ):
            identity = cp.tile([128, 128], f32)
            make_identity(nc, identity[:])
            w1af = cp.tile([64, 64], f32)
            nc.sync.dma_start(out=w1af[:], in_=w1afp[:, :])
            w1bf = cp.tile([64, 64], f32)
            nc.sync.dma_start(out=w1bf[:], in_=w1bfp[:, :])
            w1cf = cp.tile([64, 64], f32)
            nc.sync.dma_start(out=w1cf[:], in_=w1cfp[:, :])
            w1ab = cp.tile([64, 64], bf16)
            nc.sync.dma_start(out=w1ab[:], in_=w1abp[:, :])
            w1bb = cp.tile([64, 64], bf16)
            nc.sync.dma_start(out=w1bb[:], in_=w1bbp[:, :])
            w2d = cp.tile([128, 2], bf16)
            nc.sync.dma_start(out=w2d[:], in_=w2dp[:, :])
            b1t = cp.tile([1, 64], f32)
            nc.sync.dma_start(out=b1t[:], in_=b1rp[:, :])
            e5t = cp.tile([64, 1], f32)
            nc.sync.dma_start(out=e5t[:], in_=e5p[:, :])
            iot = cp.tile([128, 128], f32)
            nc.sync.dma_start(out=iot[:], in_=iotp[:, :])
            ones128 = cp.tile([1, 128], f32)
            nc.vector.memset(ones128[:], 1.0)
            noiset = cp.tile([128, SBT], f32)
            nc.sync.dma_start(out=noiset[:], in_=noisep[:, :])
            sidxt = cp.tile([128, NCHUNK * SBT], i16)
            nc.sync.dma_start(out=sidxt[:], in_=sidxp[:, :])
            if have_fx:
                fmaskt = cp.tile([128, FSB], bf16)
                nc.sync.dma_start(out=fmaskt[:], in_=fmaskp[:, :])
                fxadjt = cp.tile([128, FT], f32)
                nc.sync.dma_start(out=fxadjt[:], in_=fxadjp[:, :])
                fxcmt = cp.tile([128, FT], f32)
                nc.sync.dma_start(out=fxcmt[:], in_=fxcmp[:, :])
                fxsit = cp.tile([128, 8 * NFX], i16)
                nc.sync.dma_start(out=fxsit[:], in_=fxsip[:, :])
            sfat = cp.tile([128, SBT], f32)
            gatebf = cp.tile([128, SBT], bf16)

            cps = ppC.tile([1, 64], f32, tag="cps")
            nc.tensor.matmul(cps[:], lhsT=e5t[:], rhs=w1cf[:], start=True, stop=True)
            crow = cp.tile([1, 64], f32)
            nc.vector.tensor_tensor(out=crow[:], in0=cps[:], in1=b1t[:], op=add)

            for b in range(NBLK):
                # dest-side tables PAT/PBT [64, 128] = (Eblk @ W1x + c)^T
                et = bp.tile([128, 64], f32, tag="et")
                nc.sync.dma_start(out=et[:], in_=embp[b * 128 : (b + 1) * 128, :])
                tps = ppC.tile([64, 128], f32, tag="blkps")
                nc.tensor.transpose(tps[:, :], et[:, :], identity[:, :])
                ebT = bp.tile([64, 128], f32, tag="ebT")
                nc.scalar.copy(out=ebT[:], in_=tps[:])
                patp = ppC.tile([64, 128], f32, tag="blkps")
                nc.tensor.matmul(patp[:], lhsT=w1af[:], rhs=ebT[:], start=True, stop=False)
                nc.tensor.matmul(patp[:], lhsT=crow[:], rhs=ones128[:], start=False, stop=True)
                patf = bp.tile([64, 128], f32, tag="patf")
                nc.scalar.copy(out=patf[:], in_=patp[:])
                pbtp = ppC.tile([64, 128], f32, tag="blkps")
                nc.tensor.matmul(pbtp[:], lhsT=w1bf[:], rhs=ebT[:], start=True, stop=False)
                nc.tensor.matmul(pbtp[:], lhsT=crow[:], rhs=ones128[:], start=False, stop=True)
                pbtf = bp.tile([64, 128], f32, tag="pbtf")
                nc.scalar.copy(out=pbtf[:], in_=pbtp[:])

                S_b = int(Ss[b])
                L = 128 * S_b
                g = max(min(512 // S_b, 128), 1)
                oo = int(oT[b])

                egt1 = bp.tile([64, L], bf16, tag="egt1")
                nc.sync.dma_start(out=egt1[:], in_=egt1p[:, oo : oo + L])
                egt2 = bp.tile([64, L], bf16, tag="egt2")
                nc.sync.dma_start(out=egt2[:], in_=egt2p[:, oo : oo + L])
                pre = bp.tile([128, L], bf16, tag="pre")
                for st in (0, 1):
                    egt = egt1 if st == 0 else egt2
                    wmat = w1bb if st == 0 else w1ab
                    pt = patf if st == 0 else pbtf
                    h0 = 64 * st
                    p0 = 0
                    while p0 < 128:
                        gg = min(g, 128 - p0)
                        Lc = gg * S_b
                        c0 = p0 * S_b
                        pps = ppA.tile([64, Lc], f32, tag="pps")
                        nc.tensor.matmul(
                            pps[:], lhsT=wmat[:], rhs=egt[:, c0 : c0 + Lc],
                            start=True, stop=True,
                        )
                        pt_b = (
                            pt[:, p0 : p0 + gg]
                            .rearrange("h (g o) -> h g o", o=1)
                            .to_broadcast([64, gg, S_b])
                        )
                        nc.vector.tensor_tensor(
                            out=pre[h0 : h0 + 64, c0 : c0 + Lc].rearrange(
                                "h (g s) -> h g s", s=S_b
                            ),
                            in0=pps[:].rearrange("h (g s) -> h g s", s=S_b),
                            in1=pt_b,
                            op=add,
                        )
                        p0 += gg
                nc.scalar.activation(out=pre[:], in_=pre[:], func=AF.Relu)
                srow = srp.tile([2, L], f32, tag="srow")
                ci = 0
                p0 = 0
                while p0 < 128:
                    gg = min(g, 128 - p0)
                    Lc = gg * S_b
                    c0 = p0 * S_b
                    sps = ppB.tile([2, Lc], f32, tag="sps")
                    nc.tensor.matmul(
                        sps[:], lhsT=w2d[:], rhs=pre[:, c0 : c0 + Lc],
                        start=True, stop=True,
                    )
                    if ci % 2 == 0:
                        nc.scalar.copy(out=srow[:, c0 : c0 + Lc], in_=sps[:])
                    else:
                        nc.vector.tensor_scalar_add(srow[:, c0 : c0 + Lc], sps[:], 0.0)
                    ci += 1
                    p0 += gg
                # DRAM round-trip reshape [2, L] -> two [128, S_b] halves
                nc.sync.dma_start(out=sdram[2 * b : 2 * b + 2, 0:L], in_=srow[:])
                cdst = int(colb[b])
                for st in (0, 1):
                    nc.sync.dma_start(
                        out=sfat[:, cdst + st * S_b : cdst + st * S_b + S_b],
                        in_=sdram[2 * b + st : 2 * b + st + 1, 0:L].rearrange(
                            "o (p s) -> (o p) s", p=128
                        ),
                    )

                # gate math on fat slice [128, SB_b]
                sb = int(SB[b])
                c0 = int(colb[b])
                nzs = noiset[:, c0 : c0 + sb]
                om = sp.tile([128, sb], f32, tag="om")
                nc.vector.tensor_scalar(
                    out=om[:], in0=nzs, scalar1=-1.0, scalar2=1.0,
                    op0=mult, op1=add,
                )
                ln1 = sp.tile([128, sb], f32, tag="ln1")
                nc.scalar.activation(out=ln1[:], in_=nzs, func=AF.Ln)
                ln2 = sp.tile([128, sb], f32, tag="ln2")
                nc.scalar.activation(out=ln2[:], in_=om[:], func=AF.Ln)
                z = sp.tile([128, sb], f32, tag="z")
                nc.vector.scalar_tensor_tensor(
                    out=z[:], in0=ln1[:], scalar=b2f, in1=ln2[:],
                    op0=add, op1=subtract,
                )
                nc.vector.tensor_tensor(
                    out=z[:], in0=z[:], in1=sfat[:, c0 : c0 + sb], op=add
                )
                gf = sp.tile([128, sb], f32, tag="gf")
                nc.scalar.activation(out=gf[:], in_=z[:], func=AF.Sigmoid)
                nc.vector.tensor_scalar_mul(gatebf[:, c0 : c0 + sb], gf[:], 0.5)

                # dense mask chunks + multiply + writeout
                for j in range(NCHUNK):
                    adjt = wp.tile([128, CHW], bf16, tag="adjt")
                    nc.sync.dma_start(
                        out=adjt[:],
                        in_=adjp[b * 128 : (b + 1) * 128, j * CHW : (j + 1) * CHW],
                    )
                    mask = wp.tile([128, CHW], bf16, tag="mask")
                    nc.gpsimd.local_scatter(
                        out_ap=mask[:],
                        data_ap=gatebf[:, c0 : c0 + sb],
                        idxs_ap=sidxt[
                            :,
                            int(sidx_off[b]) + j * sb : int(sidx_off[b])
                            + (j + 1) * sb,
                        ],
                        channels=128,
                        num_elems=CHW,
                        num_idxs=sb,
                    )
                    nc.vector.tensor_tensor(
                        out=mask[:], in0=mask[:], in1=adjt[:], op=mult
                    )
                    nc.sync.dma_start(
                        out=outp[b * 128 : (b + 1) * 128, j * CHW : (j + 1) * CHW],
                        in_=mask[:],
                    )

                # fixup scatter-adds (rare duplicate cells)
                for r in range(int(Fs[b])):
                    fm = fmaskt[
                        :, int(fmoff[b]) + r * sb : int(fmoff[b]) + (r + 1) * sb
                    ]
                    prod = sp.tile([128, sb], bf16, tag="fprod")
                    nc.vector.tensor_tensor(
                        out=prod[:], in0=gatebf[:, c0 : c0 + sb], in1=fm, op=mult
                    )
                    fxg = sp.tile([128, 1], f32, tag="fxg")
                    nc.vector.tensor_reduce(
                        out=fxg[:], in_=prod[:], axis=mybir.AxisListType.X, op=add
                    )
                    t0 = sp.tile([128, 1], f32, tag="t0")
                    nc.vector.tensor_tensor(
                        out=t0[:], in0=fxg[:],
                        in1=fxadjt[:, int(foff[b]) + r : int(foff[b]) + r + 1],
                        op=mult,
                    )
                    oh = sp.tile([128, 128], f32, tag="oh")
                    nc.vector.tensor_tensor(
                        out=oh[:], in0=iot[:],
                        in1=fxcmt[
                            :, int(foff[b]) + r : int(foff[b]) + r + 1
                        ].to_broadcast([128, 128]),
                        op=is_equal,
                    )
                    pay = sp.tile([128, 128], bf16, tag="pay")
                    nc.vector.tensor_tensor(
                        out=pay[:], in0=oh[:], in1=t0[:].to_broadcast([128, 128]),
                        op=mult,
                    )
                    grp = int(foff[b]) + r
                    out_view = outp[b * 128 : (b + 1) * 128, :].rearrange(
                        "p (s w) -> (p s) w", w=SEGW
                    )
                    nc.gpsimd.dma_scatter_add(
                        out_ap=out_view,
                        in_ap=pay[:].rearrange("p (s d) -> p s d", d=SEGW),
                        idxs_ap=fxsit[:, grp * 8 : (grp + 1) * 8],
                        num_idxs=128,
                        num_idxs_reg=128,
                        elem_size=SEGW,
                    )

    nc.compile()
    return nc


def kernel(embed, row, col, adj, noise, W1, b1, W2, b2, node_idx):
    from concourse.bass_utils import run_bass_kernel_spmd

    embed = np.ascontiguousarray(np.asarray(embed), dtype=np.float32)
    adj = np.ascontiguousarray(np.asarray(adj), dtype=np.float32)
    W1 = np.ascontiguousarray(np.asarray(W1), dtype=np.float32)
    b1 = np.ascontiguousarray(np.asarray(b1), dtype=np.float32).ravel()
    W2 = np.ascontiguousarray(np.asarray(W2), dtype=np.float32)
    b2f = float(np.asarray(b2, dtype=np.float32).ravel()[0])
    nidx = int(np.asarray(node_idx))

    per_core, orders, meta = _prep_host(row, col, noise, adj, embed)
    nc = _build_program(meta, b2f)

    w1a = np.ascontiguousarray(W1[0:64])
    w1b = np.ascontiguousarray(W1[64:128])
    w1c = np.ascontiguousarray(W1[128:192])
    w2v = W2.reshape(-1)
    w2d = np.zeros((128, 2), np.float32)
    w2d[:64, 0] = w2v
    w2d[64:, 1] = w2v
    common = dict(
        e5=np.ascontiguousarray(embed[nidx].reshape(64, 1)),
        w1af=w1a, w1bf=w1b, w1cf=w1c,
        w1ab=np.ascontiguousarray(w1a.astype(BF16)),
        w1bb=np.ascontiguousarray(w1b.astype(BF16)),
        w2d=np.ascontiguousarray(w2d.astype(BF16)),
        b1r=np.ascontiguousarray(b1.reshape(1, 64)),
        iot=np.ascontiguousarray(
            np.tile(np.arange(128, dtype=np.float32), (128, 1))
        ),
    )
    in_maps = []
    for k in range(NCORES):
        mcore = dict(per_core[k])
        mcore.update(common)
        in_maps.append(mcore)

    res = run_bass_kernel_spmd(nc, in_maps, list(range(NCORES)))
    kernel.last_exec_time_ns = res.exec_time_ns
    it = getattr(res, "instructions_and_trace", None)
    kernel.last_trace_path = it[1] if it else None

    bp_index = (
        np.repeat(np.arange(NBLK), RPB) * 128 + np.tile(np.arange(RPB), NBLK)
    )
    out = np.empty((N, N), np.float32)
    for k in range(NCORES):
        o = np.asarray(res.results[k]["out"])[:, :N].astype(np.float32)
        out[orders[k] + k * RPC] = o[bp_index]
    return out


kernel.last_exec_time_ns = None
kernel.last_trace_path = None


# revision 8
# speedup vs baseline: 9.3213x; 1.4576x over previous
"""Trainium2 Bass kernel for the GNN ExplainModule (masked adjacency).

Dense row-block design (8 NeuronCores, row-sharded output):
  - Core k owns rows [k*1250, (k+1)*1250). Rows are re-ordered by token
    count and grouped into 10 blocks of 125 rows (partitions 0-124).
  - Every mask contribution ("token") for cell (r, c) lives in the
    partition of its dest row r. Two streams per block share one slot
    grid of width S_b: stream1 = copy1 tokens (dest side uses W1a) on
    hidden partitions 0-63, stream2 = copy2 (dest side uses W1b) on
    partitions 64-127.
  - MLP runs in transposed layout [hidden x tokens]: host pre-gathers
    embed[c] columns (bf16) per token; PE computes (E @ W1x)^T per
    <=512-token chunk into a [128, L] pre tile (both streams stacked);
    the dest-row term (Eblk @ W1y + c)^T is added via a partition-run
    broadcast view; one relu covers both streams; a single PE matvec
    with the block-diagonal [w2;0|0;w2] weight yields both streams'
    logits [2, L]; a DRAM round-trip reshapes them into the fat
    [128, S] layout where the concrete gate is computed.
  - gpsimd local_scatter turns each block's gates into a dense
    [128, 2000] mask chunk; DVE multiplies by the adj chunk (bf16); the
    product is written out densely. Duplicate cells (same (r,c) fed by
    several edges) keep the first token in the dense path; the rare
    followers (~300/core) are applied afterwards with per-rank
    dma_scatter_add CCE adds of one-hot payloads.
"""

import sys

import numpy as np

for _p in ("/opt/trn_rl_repo",):
    if _p not in sys.path:
        sys.path.insert(0, _p)

import ml_dtypes

BF16 = ml_dtypes.bfloat16

N = 10000
D = 64
NCORES = 8
RPC = N // NCORES  # 1250 rows per core
NBLK = 10
RPB = RPC // NBLK  # 125 real rows per block
COLS = 10000
NCHUNK = 5
CHW = 2000  # dense chunk width


def _group_rank(key):
    """Rank of each element within its key-group (appearance order)."""
    o = np.argsort(key, kind="stable")
    ks = key[o]
    starts = np.flatnonzero(np.concatenate([[True], ks[1:] != ks[:-1]]))
    sizes = np.diff(np.concatenate([starts, [len(ks)]]))
    rank_sorted = np.arange(len(ks)) - np.repeat(starts, sizes)
    rank = np.empty(len(key), np.int64)
    rank[o] = rank_sorted
    return rank


def _prep_host(row, col, noise, adj, embed):
    """Route tokens, balance rows into blocks, build all per-core arrays."""
    row = np.asarray(row).astype(np.int64).ravel()
    col = np.asarray(col).astype(np.int64).ravel()
    noise = np.asarray(noise).astype(np.float32).ravel()
    adj = np.asarray(adj, dtype=np.float32)
    embed = np.asarray(embed, dtype=np.float32)
    embed_bf = embed.astype(BF16)

    E = row.shape[0]
    t_r = np.concatenate([row, col])  # dest row
    t_c = np.concatenate([col, row])  # dest col == other endpoint
    t_nz = np.concatenate([noise, noise])
    t_st = np.concatenate([np.zeros(E, np.int8), np.ones(E, np.int8)])
    core_of = t_r // RPC

    # ---- pass 1: per-core row stats and block structure ----
    per_core_tok = []
    orders = []
    Sm = np.zeros((NCORES, NBLK), np.int64)  # shared slot width per block
    for k in range(NCORES):
        m = core_of == k
        r_loc = (t_r[m] - k * RPC).astype(np.int64)
        cc = t_c[m].astype(np.int64)
        nz = t_nz[m]
        st = t_st[m].astype(np.int64)
        n1 = np.bincount(r_loc[st == 0], minlength=RPC)
        n2 = np.bincount(r_loc[st == 1], minlength=RPC)
        nm = np.maximum(n1, n2)
        order = np.argsort(-nm, kind="stable")
        orders.append(order)
        blk_of_row = np.empty(RPC, np.int64)
        part_of_row = np.empty(RPC, np.int64)
        for b in range(NBLK):
            rows_b = order[b * RPB : (b + 1) * RPB]
            blk_of_row[rows_b] = b
            part_of_row[rows_b] = np.arange(RPB)
            Sm[k, b] = max(int(nm[rows_b].max()), 1)
        per_core_tok.append((r_loc, cc, nz, st, blk_of_row, part_of_row))

    # SPMD-static shapes: max over cores
    Ss = Sm.max(axis=0)  # shared per-stream slot count per block
    SB = 2 * Ss  # fat width per block (even by construction)

    colb = np.concatenate([[0], np.cumsum(SB)]).astype(np.int64)
    SBT = int(colb[-1])
    oT = np.concatenate([[0], np.cumsum(128 * Ss)]).astype(np.int64)
    TT = int(oT[-1])  # per-stream token columns
    sidx_off = np.concatenate([[0], np.cumsum(NCHUNK * SB)]).astype(np.int64)

    # ---- pass 2: slots, duplicates, fixup ranks ----
    staged = []
    F = np.zeros((NCORES, NBLK), np.int64)
    for k in range(NCORES):
        r_loc, cc, nz, st, blk_of_row, part_of_row = per_core_tok[k]
        b_s = blk_of_row[r_loc]
        p_s = part_of_row[r_loc]
        key = ((b_s * 128 + p_s) * 2 + st) * N + cc
        o = np.argsort(key, kind="stable")
        b_s, p_s, c_s, nz_s, st_s = b_s[o], p_s[o], cc[o], nz[o], st[o]
        slot = _group_rank((b_s * 128 + p_s) * 2 + st_s)
        fat = np.where(st_s == 0, slot, Ss[b_s] + slot)
        crank = _group_rank((b_s * 128 + p_s) * N + c_s)
        is_fol = crank > 0
        frank = np.full(len(b_s), -1, np.int64)
        fi = np.flatnonzero(is_fol)
        if len(fi):
            frank[fi] = _group_rank(b_s[fi] * 128 + p_s[fi])
            for b in range(NBLK):
                mb = b_s[fi] == b
                F[k, b] = int(frank[fi][mb].max()) + 1 if mb.any() else 0
        staged.append(dict(b=b_s, p=p_s, c=c_s, nz=nz_s, st=st_s, fat=fat,
                           fol=fi, frank=frank))

    Fs = F.max(axis=0)
    foff = np.concatenate([[0], np.cumsum(Fs)]).astype(np.int64)
    fmoff = np.concatenate([[0], np.cumsum(Fs * SB)]).astype(np.int64)
    FT = max(int(foff[-1]), 1)
    FSB = max(int(fmoff[-1]), 1)
    NFX = max(int(Fs.sum()), 1)

    meta = dict(
        Ss=Ss, SB=SB, Fs=Fs, colb=colb, oT=oT,
        sidx_off=sidx_off, foff=foff, fmoff=fmoff,
        SBT=SBT, TT=TT, FT=FT, FSB=FSB, NFX=NFX,
    )

    bp_index = (
        np.repeat(np.arange(NBLK), RPB) * 128 + np.tile(np.arange(RPB), NBLK)
    )

    per_core = []
    for k in range(NCORES):
        s = staged[k]
        b_s, p_s, c_s, nz_s, st_s, fat = (
            s["b"], s["p"], s["c"], s["nz"], s["st"], s["fat"],
        )
        fi, frank = s["fol"], s["frank"]
        is_fol = np.zeros(len(b_s), bool)
        is_fol[fi] = True
        order = orders[k]

        egt1 = np.zeros((64, TT), BF16)
        egt2 = np.zeros((64, TT), BF16)
        noisef = np.full((128, SBT), 0.5, np.float32)
        sidx = np.full((128, NCHUNK * SBT), -1, np.int16)
        fmask = np.zeros((128, FSB), BF16)
        leadm = np.zeros((128, FSB), np.float32)

        st1 = st_s == 0
        colx = oT[b_s] + p_s * Ss[b_s] + np.where(st1, fat, fat - Ss[b_s])
        egt1[:, colx[st1]] = embed_bf[c_s[st1]].T
        egt2[:, colx[~st1]] = embed_bf[c_s[~st1]].T
        noisef[p_s, colb[b_s] + fat] = nz_s
        keep = ~is_fol
        j = c_s // CHW
        sidx[
            p_s[keep],
            sidx_off[b_s[keep]] + j[keep] * SB[b_s[keep]] + fat[keep],
        ] = (c_s[keep] - j[keep] * CHW).astype(np.int16)

        if len(fi):
            # leader fat slot per cell group (first element in group order)
            cell = (b_s * 128 + p_s) * N + c_s
            co = np.argsort(cell, kind="stable")
            cs_ = cell[co]
            starts = np.flatnonzero(
                np.concatenate([[True], cs_[1:] != cs_[:-1]])
            )
            sizes = np.diff(np.concatenate([starts, [len(cs_)]]))
            lead_fat = np.empty(len(cell), np.int64)
            lead_fat[co] = np.repeat(fat[co][starts], sizes)
            fb, fp, fr = b_s[fi], p_s[fi], frank[fi]
            fmask[fp, fmoff[fb] + fr * SB[fb] + fat[fi]] = 1
            leadm[fp, fmoff[fb] + fr * SB[fb] + lead_fat[fi]] = 1.0

        adjp = np.zeros((NBLK * 128, COLS), BF16)
        embp = np.zeros((NBLK * 128, 64), np.float32)
        rows_g = order + k * RPC
        adjp[bp_index, :N] = adj[rows_g].astype(BF16)
        embp[bp_index] = embed[rows_g]

        per_core.append(
            dict(
                egt1=egt1, egt2=egt2, noisef=noisef, sidx=sidx,
                fmask=fmask, leadm=leadm, adjp=adjp, embp=embp,
            )
        )
    return per_core, orders, meta


def _emulate_core(m, meta, W1, b1, W2, b2):
    """Numpy emulation of the device program for one core (testing aid)."""
    Ss, SB = meta["Ss"], meta["SB"]
    Fs, colb, oT = meta["Fs"], meta["colb"], meta["oT"]
    sidx_off, foff, fmoff = meta["sidx_off"], meta["foff"], meta["fmoff"]

    W1a = W1[0:64].astype(np.float32)
    W1b = W1[64:128].astype(np.float32)
    w2 = W2.reshape(-1).astype(BF16).astype(np.float32)
    W1ab = W1a.astype(BF16).astype(np.float32)
    W1bb = W1b.astype(BF16).astype(np.float32)
    crow = m["_crow"]

    out = np.zeros((NBLK * 128, COLS), np.float32)
    egt1 = m["egt1"].astype(np.float32)
    egt2 = m["egt2"].astype(np.float32)
    embp = m["embp"]

    sfat = np.zeros((128, meta["SBT"]), np.float32)
    for b in range(NBLK):
        Eblk = embp[b * 128 : (b + 1) * 128]
        PAT = (Eblk @ W1a + crow).T
        PBT = (Eblk @ W1b + crow).T
        S_b = Ss[b]
        L = 128 * S_b
        pre1 = W1bb.T @ egt1[:, oT[b] : oT[b] + L] + np.repeat(PAT, S_b, axis=1)
        pre2 = W1ab.T @ egt2[:, oT[b] : oT[b] + L] + np.repeat(PBT, S_b, axis=1)
        pre1 = np.maximum(pre1.astype(BF16).astype(np.float32), 0.0)
        pre2 = np.maximum(pre2.astype(BF16).astype(np.float32), 0.0)
        s1 = (w2 @ pre1).reshape(128, S_b)
        s2 = (w2 @ pre2).reshape(128, S_b)
        sfat[:, colb[b] : colb[b] + S_b] = s1
        sfat[:, colb[b] + S_b : colb[b] + SB[b]] = s2

    nz = m["noisef"]
    z = np.log(nz) - np.log1p(-nz) + sfat + float(b2)
    gate = 1.0 / (1.0 + np.exp(-z))
    gatebf = (gate * 0.5).astype(BF16)

    for b in range(NBLK):
        gsl = gatebf[:, colb[b] : colb[b] + SB[b]]
        for r in range(Fs[b]):
            fm = m["fmask"][:, fmoff[b] + r * SB[b] : fmoff[b] + (r + 1) * SB[b]]
            lm = m["leadm"][:, fmoff[b] + r * SB[b] : fmoff[b] + (r + 1) * SB[b]]
            famt = (gsl.astype(np.float32) * fm.astype(np.float32)).sum(
                axis=1, keepdims=True
            )
            tl = (famt * lm).astype(BF16)
            gsl[:] = (gsl.astype(np.float32) + tl.astype(np.float32)).astype(
                BF16
            )
        for j in range(NCHUNK):
            idx = m["sidx"][
                :, sidx_off[b] + j * SB[b] : sidx_off[b] + (j + 1) * SB[b]
            ]
            mask = np.zeros((128, CHW), BF16)
            rows, cols_ = np.where(idx >= 0)
            mask[rows, idx[rows, cols_]] = gsl[rows, cols_]
            prod = (
                m["adjp"][b * 128 : (b + 1) * 128, j * CHW : (j + 1) * CHW]
                * mask
            ).astype(BF16)
            out[b * 128 : (b + 1) * 128, j * CHW : (j + 1) * CHW] = prod
    return out


def _build_program(meta, b2f):
    import concourse.bacc as bacc
    import concourse.mybir as mybir
    import concourse.tile as tile
    from concourse.masks import make_identity

    f32 = mybir.dt.float32
    bf16 = mybir.dt.bfloat16
    i16 = mybir.dt.int16
    add = mybir.AluOpType.add
    mult = mybir.AluOpType.mult
    subtract = mybir.AluOpType.subtract
    is_equal = mybir.AluOpType.is_equal
    AF = mybir.ActivationFunctionType

    Ss, SB = meta["Ss"], meta["SB"]
    Fs, colb, oT = meta["Fs"], meta["colb"], meta["oT"]
    sidx_off, foff, fmoff = meta["sidx_off"], meta["foff"], meta["fmoff"]
    SBT, TT, FT, FSB, NFX = (
        meta["SBT"], meta["TT"], meta["FT"], meta["FSB"], meta["NFX"],
    )
    have_fx = int(Fs.sum()) > 0
    LMAX = 128 * int(Ss.max())

    nc = bacc.Bacc()

    egt1p = nc.declare_dram_parameter("egt1", [64, TT], bf16, isOutput=False)
    egt2p = nc.declare_dram_parameter("egt2", [64, TT], bf16, isOutput=False)
    noisep = nc.declare_dram_parameter("noisef", [128, SBT], f32, isOutput=False)
    sidxp = nc.declare_dram_parameter("sidx", [128, NCHUNK * SBT], i16, isOutput=False)
    fmaskp = nc.declare_dram_parameter("fmask", [128, FSB], bf16, isOutput=False)
    leadmp = nc.declare_dram_parameter("leadm", [128, FSB], f32, isOutput=False)
    adjp = nc.declare_dram_parameter("adjp", [NBLK * 128, COLS], bf16, isOutput=False)
    embp = nc.declare_dram_parameter("embp", [NBLK * 128, 64], f32, isOutput=False)
    e5p = nc.declare_dram_parameter("e5", [64, 1], f32, isOutput=False)
    w1afp = nc.declare_dram_parameter("w1af", [64, 64], f32, isOutput=False)
    w1bfp = nc.declare_dram_parameter("w1bf", [64, 64], f32, isOutput=False)
    w1cfp = nc.declare_dram_parameter("w1cf", [64, 64], f32, isOutput=False)
    w1abp = nc.declare_dram_parameter("w1ab", [64, 64], bf16, isOutput=False)
    w1bbp = nc.declare_dram_parameter("w1bb", [64, 64], bf16, isOutput=False)
    w2dp = nc.declare_dram_parameter("w2d", [128, 2], bf16, isOutput=False)
    b1rp = nc.declare_dram_parameter("b1r", [1, 64], f32, isOutput=False)
    outp = nc.declare_dram_parameter("out", [NBLK * 128, COLS], bf16, isOutput=True)

    sdram = nc.dram_tensor("sdram", [2 * NBLK, LMAX], f32)

    with tile.TileContext(nc) as tc:
        with (
            tc.tile_pool(name="const", bufs=1) as cp,
            tc.tile_pool(name="blk", bufs=2) as bp,
            tc.tile_pool(name="srowp", bufs=1) as srp,
            tc.tile_pool(name="work", bufs=3) as wp,
            tc.tile_pool(name="small", bufs=2) as sp,
            tc.tile_pool(name="psA", bufs=4, space="PSUM") as ppA,
            tc.tile_pool(name="psB", bufs=2, space="PSUM") as ppB,
            tc.tile_pool(name="psC", bufs=1, space="PSUM") as ppC,
        ):
            identity = cp.tile([128, 128], f32)
            make_identity(nc, identity[:])
            w1af = cp.tile([64, 64], f32)
            nc.sync.dma_start(out=w1af[:], in_=w1afp[:, :])
            w1bf = cp.tile([64, 64], f32)
            nc.sync.dma_start(out=w1bf[:], in_=w1bfp[:, :])
            w1cf = cp.tile([64, 64], f32)
            nc.sync.dma_start(out=w1cf[:], in_=w1cfp[:, :])
            w1ab = cp.tile([64, 64], bf16)
            nc.sync.dma_start(out=w1ab[:], in_=w1abp[:, :])
            w1bb = cp.tile([64, 64], bf16)
            nc.sync.dma_start(out=w1bb[:], in_=w1bbp[:, :])
            w2d = cp.tile([128, 2], bf16)
            nc.sync.dma_start(out=w2d[:], in_=w2dp[:, :])
            b1t = cp.tile([1, 64], f32)
            nc.sync.dma_start(out=b1t[:], in_=b1rp[:, :])
            e5t = cp.tile([64, 1], f32)
            nc.sync.dma_start(out=e5t[:], in_=e5p[:, :])
            ones128 = cp.tile([1, 128], f32)
            nc.vector.memset(ones128[:], 1.0)
            noiset = cp.tile([128, SBT], f32)
            nc.sync.dma_start(out=noiset[:], in_=noisep[:, :])
            sidxt = cp.tile([128, NCHUNK * SBT], i16)
            nc.sync.dma_start(out=sidxt[:], in_=sidxp[:, :])
            if have_fx:
                fmaskt = cp.tile([128, FSB], bf16)
                nc.sync.dma_start(out=fmaskt[:], in_=fmaskp[:, :])
                leadmt = cp.tile([128, FSB], f32)
                nc.sync.dma_start(out=leadmt[:], in_=leadmp[:, :])
            sfat = cp.tile([128, SBT], f32)
            gatebf = cp.tile([128, SBT], bf16)
            zpre = cp.tile([128, SBT], f32)

            cps = ppC.tile([1, 64], f32, tag="cps")
            nc.tensor.matmul(cps[:], lhsT=e5t[:], rhs=w1cf[:], start=True, stop=True)
            crow = cp.tile([1, 64], f32)
            nc.vector.tensor_tensor(out=crow[:], in0=cps[:], in1=b1t[:], op=add)

            # noise logit for all blocks: zpre = ln(nz) + b2 - ln(1 - nz)
            omA = cp.tile([128, SBT], f32)
            nc.vector.tensor_scalar(
                out=omA[:], in0=noiset[:], scalar1=-1.0, scalar2=1.0,
                op0=mult, op1=add,
            )
            ln1A = cp.tile([128, SBT], f32)
            nc.scalar.activation(out=ln1A[:], in_=noiset[:], func=AF.Ln)
            nc.scalar.activation(out=omA[:], in_=omA[:], func=AF.Ln)
            nc.vector.scalar_tensor_tensor(
                out=zpre[:], in0=ln1A[:], scalar=b2f, in1=omA[:],
                op0=add, op1=subtract,
            )

            for b in range(NBLK):
                # dest-side tables PAT/PBT [64, 128] = (Eblk @ W1x + c)^T
                et = bp.tile([128, 64], f32, tag="et")
                nc.sync.dma_start(out=et[:], in_=embp[b * 128 : (b + 1) * 128, :])
                tps = ppC.tile([64, 128], f32, tag="blkps")
                nc.tensor.transpose(tps[:, :], et[:, :], identity[:, :])
                ebT = bp.tile([64, 128], f32, tag="ebT")
                nc.scalar.copy(out=ebT[:], in_=tps[:])
                patp = ppC.tile([64, 128], f32, tag="blkps")
                nc.tensor.matmul(patp[:], lhsT=w1af[:], rhs=ebT[:], start=True, stop=False)
                nc.tensor.matmul(patp[:], lhsT=crow[:], rhs=ones128[:], start=False, stop=True)
                patf = bp.tile([64, 128], f32, tag="patf")
                nc.scalar.copy(out=patf[:], in_=patp[:])
                pbtp = ppC.tile([64, 128], f32, tag="blkps")
                nc.tensor.matmul(pbtp[:], lhsT=w1bf[:], rhs=ebT[:], start=True, stop=False)
                nc.tensor.matmul(pbtp[:], lhsT=crow[:], rhs=ones128[:], start=False, stop=True)
                pbtf = bp.tile([64, 128], f32, tag="pbtf")
                nc.scalar.copy(out=pbtf[:], in_=pbtp[:])

                S_b = int(Ss[b])
                L = 128 * S_b
                g = max(min(512 // S_b, 128), 1)
                oo = int(oT[b])

                egt1 = bp.tile([64, L], bf16, tag="egt1")
                nc.sync.dma_start(out=egt1[:], in_=egt1p[:, oo : oo + L])
                egt2 = bp.tile([64, L], bf16, tag="egt2")
                nc.sync.dma_start(out=egt2[:], in_=egt2p[:, oo : oo + L])
                pre = bp.tile([128, L], bf16, tag="pre")
                for st in (0, 1):
                    egt = egt1 if st == 0 else egt2
                    wmat = w1bb if st == 0 else w1ab
                    pt = patf if st == 0 else pbtf
                    h0 = 64 * st
                    p0 = 0
                    while p0 < 128:
                        gg = min(g, 128 - p0)
                        Lc = gg * S_b
                        c0 = p0 * S_b
                        pps = ppA.tile([64, Lc], f32, tag="pps")
                        nc.tensor.matmul(
                            pps[:], lhsT=wmat[:], rhs=egt[:, c0 : c0 + Lc],
                            start=True, stop=True,
                        )
                        pt_b = (
                            pt[:, p0 : p0 + gg]
                            .rearrange("h (g o) -> h g o", o=1)
                            .to_broadcast([64, gg, S_b])
                        )
                        nc.vector.tensor_tensor(
                            out=pre[h0 : h0 + 64, c0 : c0 + Lc].rearrange(
                                "h (g s) -> h g s", s=S_b
                            ),
                            in0=pps[:].rearrange("h (g s) -> h g s", s=S_b),
                            in1=pt_b,
                            op=add,
                        )
                        p0 += gg
                nc.scalar.activation(out=pre[:], in_=pre[:], func=AF.Relu)
                srow = srp.tile([2, L], f32, tag="srow")
                ci = 0
                p0 = 0
                while p0 < 128:
                    gg = min(g, 128 - p0)
                    Lc = gg * S_b
                    c0 = p0 * S_b
                    sps = ppB.tile([2, Lc], f32, tag="sps")
                    nc.tensor.matmul(
                        sps[:], lhsT=w2d[:], rhs=pre[:, c0 : c0 + Lc],
                        start=True, stop=True,
                    )
                    if ci % 2 == 0:
                        nc.scalar.copy(out=srow[:, c0 : c0 + Lc], in_=sps[:])
                    else:
                        nc.vector.tensor_scalar_add(srow[:, c0 : c0 + Lc], sps[:], 0.0)
                    ci += 1
                    p0 += gg
                # DRAM round-trip reshape [2, L] -> two [128, S_b] halves
                nc.sync.dma_start(out=sdram[2 * b : 2 * b + 2, 0:L], in_=srow[:])
                cdst = int(colb[b])
                for st in (0, 1):
                    nc.sync.dma_start(
                        out=sfat[:, cdst + st * S_b : cdst + st * S_b + S_b],
                        in_=sdram[2 * b + st : 2 * b + st + 1, 0:L].rearrange(
                            "o (p s) -> (o p) s", p=128
                        ),
                    )

                # gate math on fat slice [128, SB_b]
                sb = int(SB[b])
                c0 = int(colb[b])
                z = sp.tile([128, sb], f32, tag="z")
                nc.vector.tensor_tensor(
                    out=z[:], in0=zpre[:, c0 : c0 + sb],
                    in1=sfat[:, c0 : c0 + sb], op=add,
                )
                gf = sp.tile([128, sb], f32, tag="gf")
                nc.scalar.activation(out=gf[:], in_=z[:], func=AF.Sigmoid)
                nc.vector.tensor_scalar_mul(gatebf[:, c0 : c0 + sb], gf[:], 0.5)

                # fold duplicate-cell follower gates into their leader slot
                for r in range(int(Fs[b])):
                    fsl = slice(int(fmoff[b]) + r * sb, int(fmoff[b]) + (r + 1) * sb)
                    prod = sp.tile([128, sb], bf16, tag="fprod")
                    nc.vector.tensor_tensor(
                        out=prod[:], in0=gatebf[:, c0 : c0 + sb],
                        in1=fmaskt[:, fsl], op=mult,
                    )
                    famt = sp.tile([128, 1], f32, tag="famt")
                    nc.vector.tensor_reduce(
                        out=famt[:], in_=prod[:], axis=mybir.AxisListType.X,
                        op=add,
                    )
                    tl = sp.tile([128, sb], bf16, tag="tl")
                    nc.vector.tensor_tensor(
                        out=tl[:], in0=leadmt[:, fsl],
                        in1=famt[:].to_broadcast([128, sb]), op=mult,
                    )
                    nc.vector.tensor_tensor(
                        out=gatebf[:, c0 : c0 + sb],
                        in0=gatebf[:, c0 : c0 + sb], in1=tl[:], op=add,
                    )

                # dense mask chunks + multiply + writeout
                for j in range(NCHUNK):
                    adjt = wp.tile([128, CHW], bf16, tag="adjt")
                    nc.sync.dma_start(
                        out=adjt[:],
                        in_=adjp[b * 128 : (b + 1) * 128, j * CHW : (j + 1) * CHW],
                    )
                    mask = wp.tile([128, CHW], bf16, tag="mask")
                    nc.gpsimd.local_scatter(
                        out_ap=mask[:],
                        data_ap=gatebf[:, c0 : c0 + sb],
                        idxs_ap=sidxt[
                            :,
                            int(sidx_off[b]) + j * sb : int(sidx_off[b])
                            + (j + 1) * sb,
                        ],
                        channels=128,
                        num_elems=CHW,
                        num_idxs=sb,
                    )
                    nc.vector.tensor_tensor(
                        out=mask[:], in0=mask[:], in1=adjt[:], op=mult
                    )
                    nc.sync.dma_start(
                        out=outp[b * 128 : (b + 1) * 128, j * CHW : (j + 1) * CHW],
                        in_=mask[:],
                    )

    nc.compile()
    return nc


def kernel(embed, row, col, adj, noise, W1, b1, W2, b2, node_idx):
    from concourse.bass_utils import run_bass_kernel_spmd

    embed = np.ascontiguousarray(np.asarray(embed), dtype=np.float32)
    adj = np.ascontiguousarray(np.asarray(adj), dtype=np.float32)
    W1 = np.ascontiguousarray(np.asarray(W1), dtype=np.float32)
    b1 = np.ascontiguousarray(np.asarray(b1), dtype=np.float32).ravel()
    W2 = np.ascontiguousarray(np.asarray(W2), dtype=np.float32)
    b2f = float(np.asarray(b2, dtype=np.float32).ravel()[0])
    nidx = int(np.asarray(node_idx))

    per_core, orders, meta = _prep_host(row, col, noise, adj, embed)
    nc = _build_program(meta, b2f)

    w1a = np.ascontiguousarray(W1[0:64])
    w1b = np.ascontiguousarray(W1[64:128])
    w1c = np.ascontiguousarray(W1[128:192])
    w2v = W2.reshape(-1)
    w2d = np.zeros((128, 2), np.float32)
    w2d[:64, 0] = w2v
    w2d[64:, 1] = w2v
    common = dict(
        e5=np.ascontiguousarray(embed[nidx].reshape(64, 1)),
        w1af=w1a, w1bf=w1b, w1cf=w1c,
        w1ab=np.ascontiguousarray(w1a.astype(BF16)),
        w1bb=np.ascontiguousarray(w1b.astype(BF16)),
        w2d=np.ascontiguousarray(w2d.astype(BF16)),
        b1r=np.ascontiguousarray(b1.reshape(1, 64)),
    )
    in_maps = []
    for k in range(NCORES):
        mcore = dict(per_core[k])
        mcore.update(common)
        in_maps.append(mcore)

    res = run_bass_kernel_spmd(nc, in_maps, list(range(NCORES)))
    kernel.last_exec_time_ns = res.exec_time_ns
    it = getattr(res, "instructions_and_trace", None)
    kernel.last_trace_path = it[1] if it else None

    bp_index = (
        np.repeat(np.arange(NBLK), RPB) * 128 + np.tile(np.arange(RPB), NBLK)
    )
    out = np.empty((N, N), np.float32)
    for k in range(NCORES):
        o = np.asarray(res.results[k]["out"])[:, :N].astype(np.float32)
        out[orders[k] + k * RPC] = o[bp_index]
    return out


kernel.last_exec_time_ns = None
kernel.last_trace_path = None


# revision 10
# speedup vs baseline: 10.7499x; 1.1533x over previous
"""Trainium2 Bass kernel for the GNN ExplainModule (masked adjacency).

Dense row-block design (8 NeuronCores, row-sharded output):
  - Core k owns rows [k*1250, (k+1)*1250). Rows are re-ordered by token
    count and grouped into 10 blocks of 125 rows (partitions 0-124).
  - Every mask contribution ("token") for cell (r, c) lives in the
    partition of its dest row r. Two streams per block share one slot
    grid of width S_b: stream1 = copy1 tokens (dest side uses W1a) on
    hidden partitions 0-63, stream2 = copy2 (dest side uses W1b) on
    partitions 64-127.
  - MLP runs in transposed layout [hidden x tokens]: host pre-gathers
    embed[c] columns (bf16) per token; PE computes (E @ W1x)^T per
    <=512-token chunk into a [128, L] pre tile (both streams stacked);
    the dest-row term (Eblk @ W1y + c)^T is added via a partition-run
    broadcast view; one relu covers both streams; a single PE matvec
    with the block-diagonal [w2;0|0;w2] weight yields both streams'
    logits [2, L]; a DRAM round-trip reshapes them into the fat
    [128, S] layout where the concrete gate is computed.
  - gpsimd local_scatter turns each block's gates into a dense
    [128, 2000] mask chunk; DVE multiplies by the adj chunk (bf16); the
    product is written out densely. Duplicate cells (same (r,c) fed by
    several edges) keep the first token in the dense path; the rare
    followers (~300/core) are applied afterwards with per-rank
    dma_scatter_add CCE adds of one-hot payloads.
"""

import sys

import numpy as np

for _p in ("/opt/trn_rl_repo",):
    if _p not in sys.path:
        sys.path.insert(0, _p)

import ml_dtypes

BF16 = ml_dtypes.bfloat16

N = 10000
D = 64
NCORES = 8
RPC = N // NCORES  # 1250 rows per core
NBLK = 10
RPB = RPC // NBLK  # 125 real rows per block
COLS = 10000
NCHUNK = 5
CHW = 2000  # dense chunk width


def _group_rank(key):
    """Rank of each element within its key-group (appearance order)."""
    o = np.argsort(key, kind="stable")
    ks = key[o]
    starts = np.flatnonzero(np.concatenate([[True], ks[1:] != ks[:-1]]))
    sizes = np.diff(np.concatenate([starts, [len(ks)]]))
    rank_sorted = np.arange(len(ks)) - np.repeat(starts, sizes)
    rank = np.empty(len(key), np.int64)
    rank[o] = rank_sorted
    return rank


def _prep_host(row, col, noise, adj, embed):
    """Route tokens, balance rows into blocks, build all per-core arrays."""
    row = np.asarray(row).astype(np.int64).ravel()
    col = np.asarray(col).astype(np.int64).ravel()
    noise = np.asarray(noise).astype(np.float32).ravel()
    adj = np.asarray(adj, dtype=np.float32)
    embed = np.asarray(embed, dtype=np.float32)
    embed_bf = embed.astype(BF16)

    E = row.shape[0]
    t_r = np.concatenate([row, col])  # dest row
    t_c = np.concatenate([col, row])  # dest col == other endpoint
    t_nz = np.concatenate([noise, noise])
    t_st = np.concatenate([np.zeros(E, np.int8), np.ones(E, np.int8)])
    core_of = t_r // RPC

    # ---- pass 1: per-core row stats and block structure ----
    per_core_tok = []
    orders = []
    Sm = np.zeros((NCORES, NBLK), np.int64)  # shared slot width per block
    for k in range(NCORES):
        m = core_of == k
        r_loc = (t_r[m] - k * RPC).astype(np.int64)
        cc = t_c[m].astype(np.int64)
        nz = t_nz[m]
        st = t_st[m].astype(np.int64)
        n1 = np.bincount(r_loc[st == 0], minlength=RPC)
        n2 = np.bincount(r_loc[st == 1], minlength=RPC)
        nm = np.maximum(n1, n2)
        order = np.argsort(-nm, kind="stable")
        orders.append(order)
        blk_of_row = np.empty(RPC, np.int64)
        part_of_row = np.empty(RPC, np.int64)
        for b in range(NBLK):
            rows_b = order[b * RPB : (b + 1) * RPB]
            blk_of_row[rows_b] = b
            part_of_row[rows_b] = np.arange(RPB)
            Sm[k, b] = max(int(nm[rows_b].max()), 1)
        per_core_tok.append((r_loc, cc, nz, st, blk_of_row, part_of_row))

    # SPMD-static shapes: max over cores
    Ss = Sm.max(axis=0)  # shared per-stream slot count per block
    SB = 2 * Ss  # fat width per block (even by construction)

    colb = np.concatenate([[0], np.cumsum(SB)]).astype(np.int64)
    SBT = int(colb[-1])
    oT = np.concatenate([[0], np.cumsum(128 * Ss)]).astype(np.int64)
    TT = int(oT[-1])  # per-stream token columns
    sidx_off = np.concatenate([[0], np.cumsum(NCHUNK * SB)]).astype(np.int64)

    # ---- pass 2: slots, duplicates, fixup ranks ----
    staged = []
    F = np.zeros((NCORES, NBLK), np.int64)
    for k in range(NCORES):
        r_loc, cc, nz, st, blk_of_row, part_of_row = per_core_tok[k]
        b_s = blk_of_row[r_loc]
        p_s = part_of_row[r_loc]
        key = ((b_s * 128 + p_s) * 2 + st) * N + cc
        o = np.argsort(key, kind="stable")
        b_s, p_s, c_s, nz_s, st_s = b_s[o], p_s[o], cc[o], nz[o], st[o]
        slot = _group_rank((b_s * 128 + p_s) * 2 + st_s)
        fat = np.where(st_s == 0, slot, Ss[b_s] + slot)
        crank = _group_rank((b_s * 128 + p_s) * N + c_s)
        is_fol = crank > 0
        frank = np.full(len(b_s), -1, np.int64)
        fi = np.flatnonzero(is_fol)
        if len(fi):
            frank[fi] = _group_rank(b_s[fi] * 128 + p_s[fi])
            for b in range(NBLK):
                mb = b_s[fi] == b
                F[k, b] = int(frank[fi][mb].max()) + 1 if mb.any() else 0
        staged.append(dict(b=b_s, p=p_s, c=c_s, nz=nz_s, st=st_s, fat=fat,
                           fol=fi, frank=frank))

    Fs = F.max(axis=0)
    foff = np.concatenate([[0], np.cumsum(Fs)]).astype(np.int64)
    fmoff = np.concatenate([[0], np.cumsum(Fs * SB)]).astype(np.int64)
    FT = max(int(foff[-1]), 1)
    FSB = max(int(fmoff[-1]), 1)
    NFX = max(int(Fs.sum()), 1)

    meta = dict(
        Ss=Ss, SB=SB, Fs=Fs, colb=colb, oT=oT,
        sidx_off=sidx_off, foff=foff, fmoff=fmoff,
        SBT=SBT, TT=TT, FT=FT, FSB=FSB, NFX=NFX,
    )

    bp_index = (
        np.repeat(np.arange(NBLK), RPB) * 128 + np.tile(np.arange(RPB), NBLK)
    )

    per_core = []
    for k in range(NCORES):
        s = staged[k]
        b_s, p_s, c_s, nz_s, st_s, fat = (
            s["b"], s["p"], s["c"], s["nz"], s["st"], s["fat"],
        )
        fi, frank = s["fol"], s["frank"]
        is_fol = np.zeros(len(b_s), bool)
        is_fol[fi] = True
        order = orders[k]

        egtc = np.zeros((128, TT), BF16)
        noisef = np.full((128, SBT), 0.5, np.float32)
        sidx = np.full((128, NCHUNK * SBT), -1, np.int16)
        fmask = np.zeros((128, FSB), BF16)
        leadm = np.zeros((128, FSB), np.float32)

        st1 = st_s == 0
        colx = oT[b_s] + p_s * Ss[b_s] + np.where(st1, fat, fat - Ss[b_s])
        egtc[:64, colx[st1]] = embed_bf[c_s[st1]].T
        egtc[64:, colx[~st1]] = embed_bf[c_s[~st1]].T
        noisef[p_s, colb[b_s] + fat] = nz_s
        keep = ~is_fol
        j = c_s // CHW
        sidx[
            p_s[keep],
            sidx_off[b_s[keep]] + j[keep] * SB[b_s[keep]] + fat[keep],
        ] = (c_s[keep] - j[keep] * CHW).astype(np.int16)

        if len(fi):
            # leader fat slot per cell group (first element in group order)
            cell = (b_s * 128 + p_s) * N + c_s
            co = np.argsort(cell, kind="stable")
            cs_ = cell[co]
            starts = np.flatnonzero(
                np.concatenate([[True], cs_[1:] != cs_[:-1]])
            )
            sizes = np.diff(np.concatenate([starts, [len(cs_)]]))
            lead_fat = np.empty(len(cell), np.int64)
            lead_fat[co] = np.repeat(fat[co][starts], sizes)
            fb, fp, fr = b_s[fi], p_s[fi], frank[fi]
            fmask[fp, fmoff[fb] + fr * SB[fb] + fat[fi]] = 1
            leadm[fp, fmoff[fb] + fr * SB[fb] + lead_fat[fi]] = 1.0

        adjp = np.zeros((NBLK * 128, COLS), BF16)
        embp = np.zeros((NBLK * 128, 64), np.float32)
        rows_g = order + k * RPC
        adjp[bp_index, :N] = adj[rows_g].astype(BF16)
        embp[bp_index] = embed[rows_g]

        per_core.append(
            dict(
                egtc=egtc, noisef=noisef, sidx=sidx,
                fmask=fmask, leadm=leadm, adjp=adjp, embp=embp,
            )
        )
    return per_core, orders, meta


def _emulate_core(m, meta, W1, b1, W2, b2):
    """Numpy emulation of the device program for one core (testing aid)."""
    Ss, SB = meta["Ss"], meta["SB"]
    Fs, colb, oT = meta["Fs"], meta["colb"], meta["oT"]
    sidx_off, foff, fmoff = meta["sidx_off"], meta["foff"], meta["fmoff"]

    W1a = W1[0:64].astype(np.float32)
    W1b = W1[64:128].astype(np.float32)
    w2 = W2.reshape(-1).astype(BF16).astype(np.float32)
    W1ab = W1a.astype(BF16).astype(np.float32)
    W1bb = W1b.astype(BF16).astype(np.float32)
    crow = m["_crow"]

    out = np.zeros((NBLK * 128, COLS), np.float32)
    egt1 = m["egtc"][:64].astype(np.float32)
    egt2 = m["egtc"][64:].astype(np.float32)
    embp = m["embp"]

    sfat = np.zeros((128, meta["SBT"]), np.float32)
    for b in range(NBLK):
        Eblk = embp[b * 128 : (b + 1) * 128]
        PAT = (Eblk @ W1a + crow).T
        PBT = (Eblk @ W1b + crow).T
        S_b = Ss[b]
        L = 128 * S_b
        pre1 = W1bb.T @ egt1[:, oT[b] : oT[b] + L] + np.repeat(PAT, S_b, axis=1)
        pre2 = W1ab.T @ egt2[:, oT[b] : oT[b] + L] + np.repeat(PBT, S_b, axis=1)
        pre1 = np.maximum(pre1.astype(BF16).astype(np.float32), 0.0)
        pre2 = np.maximum(pre2.astype(BF16).astype(np.float32), 0.0)
        s1 = (w2 @ pre1).reshape(128, S_b)
        s2 = (w2 @ pre2).reshape(128, S_b)
        sfat[:, colb[b] : colb[b] + S_b] = s1
        sfat[:, colb[b] + S_b : colb[b] + SB[b]] = s2

    nz = m["noisef"]
    z = np.log(nz) - np.log1p(-nz) + sfat + float(b2)
    gate = 1.0 / (1.0 + np.exp(-z))
    gatebf = (gate * 0.5).astype(BF16)

    for b in range(NBLK):
        gsl = gatebf[:, colb[b] : colb[b] + SB[b]]
        for r in range(Fs[b]):
            fm = m["fmask"][:, fmoff[b] + r * SB[b] : fmoff[b] + (r + 1) * SB[b]]
            lm = m["leadm"][:, fmoff[b] + r * SB[b] : fmoff[b] + (r + 1) * SB[b]]
            famt = (gsl.astype(np.float32) * fm.astype(np.float32)).sum(
                axis=1, keepdims=True
            )
            tl = (famt * lm).astype(BF16)
            gsl[:] = (gsl.astype(np.float32) + tl.astype(np.float32)).astype(
                BF16
            )
        for j in range(NCHUNK):
            idx = m["sidx"][
                :, sidx_off[b] + j * SB[b] : sidx_off[b] + (j + 1) * SB[b]
            ]
            mask = np.zeros((128, CHW), BF16)
            rows, cols_ = np.where(idx >= 0)
            mask[rows, idx[rows, cols_]] = gsl[rows, cols_]
            prod = (
                m["adjp"][b * 128 : (b + 1) * 128, j * CHW : (j + 1) * CHW]
                * mask
            ).astype(BF16)
            out[b * 128 : (b + 1) * 128, j * CHW : (j + 1) * CHW] = prod
    return out


def _build_program(meta, b2f):
    import concourse.bacc as bacc
    import concourse.mybir as mybir
    import concourse.tile as tile
    from concourse.masks import make_identity

    f32 = mybir.dt.float32
    bf16 = mybir.dt.bfloat16
    i16 = mybir.dt.int16
    add = mybir.AluOpType.add
    mult = mybir.AluOpType.mult
    subtract = mybir.AluOpType.subtract
    is_equal = mybir.AluOpType.is_equal
    AF = mybir.ActivationFunctionType

    Ss, SB = meta["Ss"], meta["SB"]
    Fs, colb, oT = meta["Fs"], meta["colb"], meta["oT"]
    sidx_off, foff, fmoff = meta["sidx_off"], meta["foff"], meta["fmoff"]
    SBT, TT, FT, FSB, NFX = (
        meta["SBT"], meta["TT"], meta["FT"], meta["FSB"], meta["NFX"],
    )
    have_fx = int(Fs.sum()) > 0
    LMAX = 128 * int(Ss.max())

    nc = bacc.Bacc()

    egtcp = nc.declare_dram_parameter("egtc", [128, TT], bf16, isOutput=False)
    noisep = nc.declare_dram_parameter("noisef", [128, SBT], f32, isOutput=False)
    sidxp = nc.declare_dram_parameter("sidx", [128, NCHUNK * SBT], i16, isOutput=False)
    fmaskp = nc.declare_dram_parameter("fmask", [128, FSB], bf16, isOutput=False)
    leadmp = nc.declare_dram_parameter("leadm", [128, FSB], f32, isOutput=False)
    adjp = nc.declare_dram_parameter("adjp", [NBLK * 128, COLS], bf16, isOutput=False)
    embp = nc.declare_dram_parameter("embp", [NBLK * 128, 64], f32, isOutput=False)
    e5p = nc.declare_dram_parameter("e5", [64, 1], f32, isOutput=False)
    w1afp = nc.declare_dram_parameter("w1af", [64, 64], f32, isOutput=False)
    w1bfp = nc.declare_dram_parameter("w1bf", [64, 64], f32, isOutput=False)
    w1cfp = nc.declare_dram_parameter("w1cf", [64, 64], f32, isOutput=False)
    wbigp = nc.declare_dram_parameter("wbig", [128, 128], bf16, isOutput=False)
    w2dp = nc.declare_dram_parameter("w2d", [128, 2], bf16, isOutput=False)
    b1rp = nc.declare_dram_parameter("b1r", [1, 64], f32, isOutput=False)
    outp = nc.declare_dram_parameter("out", [NBLK * 128, COLS], bf16, isOutput=True)

    sdram = nc.dram_tensor("sdram", [2 * NBLK, LMAX], f32)

    with tile.TileContext(nc) as tc:
        with (
            tc.tile_pool(name="const", bufs=1) as cp,
            tc.tile_pool(name="blk", bufs=2) as bp,
            tc.tile_pool(name="srowp", bufs=1) as srp,
            tc.tile_pool(name="work", bufs=3) as wp,
            tc.tile_pool(name="small", bufs=2) as sp,
            tc.tile_pool(name="psA", bufs=4, space="PSUM") as ppA,
            tc.tile_pool(name="psB", bufs=2, space="PSUM") as ppB,
            tc.tile_pool(name="psC", bufs=1, space="PSUM") as ppC,
        ):
            identity = cp.tile([128, 128], f32)
            make_identity(nc, identity[:])
            w1af = cp.tile([64, 64], f32)
            nc.sync.dma_start(out=w1af[:], in_=w1afp[:, :])
            w1bf = cp.tile([64, 64], f32)
            nc.sync.dma_start(out=w1bf[:], in_=w1bfp[:, :])
            w1cf = cp.tile([64, 64], f32)
            nc.sync.dma_start(out=w1cf[:], in_=w1cfp[:, :])
            wbig = cp.tile([128, 128], bf16)
            nc.sync.dma_start(out=wbig[:], in_=wbigp[:, :])
            w2d = cp.tile([128, 2], bf16)
            nc.sync.dma_start(out=w2d[:], in_=w2dp[:, :])
            b1t = cp.tile([1, 64], f32)
            nc.sync.dma_start(out=b1t[:], in_=b1rp[:, :])
            e5t = cp.tile([64, 1], f32)
            nc.sync.dma_start(out=e5t[:], in_=e5p[:, :])
            ones128 = cp.tile([1, 128], f32)
            nc.vector.memset(ones128[:], 1.0)
            noiset = cp.tile([128, SBT], f32)
            nc.sync.dma_start(out=noiset[:], in_=noisep[:, :])
            sidxt = cp.tile([128, NCHUNK * SBT], i16)
            nc.sync.dma_start(out=sidxt[:], in_=sidxp[:, :])
            if have_fx:
                fmaskt = cp.tile([128, FSB], bf16)
                nc.sync.dma_start(out=fmaskt[:], in_=fmaskp[:, :])
                leadmt = cp.tile([128, FSB], f32)
                nc.sync.dma_start(out=leadmt[:], in_=leadmp[:, :])
            sfat = cp.tile([128, SBT], f32)
            gatebf = cp.tile([128, SBT], bf16)
            zpre = cp.tile([128, SBT], f32)

            cps = ppC.tile([1, 64], f32, tag="cps")
            nc.tensor.matmul(cps[:], lhsT=e5t[:], rhs=w1cf[:], start=True, stop=True)
            crow = cp.tile([1, 64], f32)
            nc.vector.tensor_tensor(out=crow[:], in0=cps[:], in1=b1t[:], op=add)

            # noise logit for all blocks: zpre = ln(nz) + b2 - ln(1 - nz)
            omA = cp.tile([128, SBT], f32)
            nc.vector.tensor_scalar(
                out=omA[:], in0=noiset[:], scalar1=-1.0, scalar2=1.0,
                op0=mult, op1=add,
            )
            ln1A = cp.tile([128, SBT], f32)
            nc.scalar.activation(out=ln1A[:], in_=noiset[:], func=AF.Ln)
            nc.scalar.activation(out=omA[:], in_=omA[:], func=AF.Ln)
            nc.vector.scalar_tensor_tensor(
                out=zpre[:], in0=ln1A[:], scalar=b2f, in1=omA[:],
                op0=add, op1=subtract,
            )

            def mlp_block(b):
                # dest-side tables PAT/PBT stacked [128, 128] = (Eblk @ W1x + c)^T
                et = bp.tile([128, 64], f32, tag="et")
                nc.sync.dma_start(out=et[:], in_=embp[b * 128 : (b + 1) * 128, :])
                tps = ppC.tile([64, 128], f32, tag="blkps")
                nc.tensor.transpose(tps[:, :], et[:, :], identity[:, :])
                ebT = bp.tile([64, 128], f32, tag="ebT")
                nc.scalar.copy(out=ebT[:], in_=tps[:])
                patpbt = bp.tile([128, 128], f32, tag="patpbt")
                patp = ppC.tile([64, 128], f32, tag="blkps")
                nc.tensor.matmul(patp[:], lhsT=w1af[:], rhs=ebT[:], start=True, stop=False)
                nc.tensor.matmul(patp[:], lhsT=crow[:], rhs=ones128[:], start=False, stop=True)
                nc.scalar.copy(out=patpbt[0:64, :], in_=patp[:])
                pbtp = ppC.tile([64, 128], f32, tag="blkps")
                nc.tensor.matmul(pbtp[:], lhsT=w1bf[:], rhs=ebT[:], start=True, stop=False)
                nc.tensor.matmul(pbtp[:], lhsT=crow[:], rhs=ones128[:], start=False, stop=True)
                nc.scalar.copy(out=patpbt[64:128, :], in_=pbtp[:])

                S_b = int(Ss[b])
                L = 128 * S_b
                g = max(min(512 // S_b, 128), 1)
                oo = int(oT[b])

                egtc = bp.tile([128, L], bf16, tag="egtc")
                nc.sync.dma_start(out=egtc[:], in_=egtcp[:, oo : oo + L])
                pre = bp.tile([128, L], bf16, tag="pre")
                p0 = 0
                while p0 < 128:
                    gg = min(g, 128 - p0)
                    Lc = gg * S_b
                    c0 = p0 * S_b
                    pps = ppA.tile([128, Lc], f32, tag="pps")
                    nc.tensor.matmul(
                        pps[:], lhsT=wbig[:], rhs=egtc[:, c0 : c0 + Lc],
                        start=True, stop=True,
                    )
                    pt_b = (
                        patpbt[:, p0 : p0 + gg]
                        .rearrange("h (g o) -> h g o", o=1)
                        .to_broadcast([128, gg, S_b])
                    )
                    nc.vector.tensor_tensor(
                        out=pre[:, c0 : c0 + Lc].rearrange(
                            "h (g s) -> h g s", s=S_b
                        ),
                        in0=pps[:].rearrange("h (g s) -> h g s", s=S_b),
                        in1=pt_b,
                        op=add,
                    )
                    p0 += gg
                nc.scalar.activation(out=pre[:], in_=pre[:], func=AF.Relu)
                srow = srp.tile([2, L], f32, tag="srow")
                p0 = 0
                while p0 < 128:
                    gg = min(g, 128 - p0)
                    Lc = gg * S_b
                    c0 = p0 * S_b
                    sps = ppB.tile([2, Lc], f32, tag="sps")
                    nc.tensor.matmul(
                        sps[:], lhsT=w2d[:], rhs=pre[:, c0 : c0 + Lc],
                        start=True, stop=True,
                    )
                    nc.scalar.copy(out=srow[:, c0 : c0 + Lc], in_=sps[:])
                    p0 += gg
                # DRAM round-trip reshape [2, L] -> two [128, S_b] halves
                nc.sync.dma_start(out=sdram[2 * b : 2 * b + 2, 0:L], in_=srow[:])
                cdst = int(colb[b])
                for st in (0, 1):
                    nc.sync.dma_start(
                        out=sfat[:, cdst + st * S_b : cdst + st * S_b + S_b],
                        in_=sdram[2 * b + st : 2 * b + st + 1, 0:L].rearrange(
                            "o (p s) -> (o p) s", p=128
                        ),
                    )

                # gate math on fat slice [128, SB_b]
                sb = int(SB[b])
                c0 = int(colb[b])
                z = sp.tile([128, sb], f32, tag="z")
                nc.vector.tensor_tensor(
                    out=z[:], in0=zpre[:, c0 : c0 + sb],
                    in1=sfat[:, c0 : c0 + sb], op=add,
                )
                gf = sp.tile([128, sb], f32, tag="gf")
                nc.scalar.activation(out=gf[:], in_=z[:], func=AF.Sigmoid)
                nc.vector.tensor_scalar_mul(gatebf[:, c0 : c0 + sb], gf[:], 0.5)

                # fold duplicate-cell follower gates into their leader slot
                for r in range(int(Fs[b])):
                    fsl = slice(int(fmoff[b]) + r * sb, int(fmoff[b]) + (r + 1) * sb)
                    prod = sp.tile([128, sb], bf16, tag="fprod")
                    nc.vector.tensor_tensor(
                        out=prod[:], in0=gatebf[:, c0 : c0 + sb],
                        in1=fmaskt[:, fsl], op=mult,
                    )
                    famt = sp.tile([128, 1], f32, tag="famt")
                    nc.vector.tensor_reduce(
                        out=famt[:], in_=prod[:], axis=mybir.AxisListType.X,
                        op=add,
                    )
                    tl = sp.tile([128, sb], bf16, tag="tl")
                    nc.vector.tensor_tensor(
                        out=tl[:], in0=leadmt[:, fsl],
                        in1=famt[:].to_broadcast([128, sb]), op=mult,
                    )
                    nc.vector.tensor_tensor(
                        out=gatebf[:, c0 : c0 + sb],
                        in0=gatebf[:, c0 : c0 + sb], in1=tl[:], op=add,
                    )

            def dense_block(b):
                sb = int(SB[b])
                c0 = int(colb[b])
                for j in range(NCHUNK):
                    adjt = wp.tile([128, CHW], bf16, tag="adjt")
                    nc.sync.dma_start(
                        out=adjt[:],
                        in_=adjp[b * 128 : (b + 1) * 128, j * CHW : (j + 1) * CHW],
                    )
                    mask = wp.tile([128, CHW], bf16, tag="mask")
                    nc.gpsimd.local_scatter(
                        out_ap=mask[:],
                        data_ap=gatebf[:, c0 : c0 + sb],
                        idxs_ap=sidxt[
                            :,
                            int(sidx_off[b]) + j * sb : int(sidx_off[b])
                            + (j + 1) * sb,
                        ],
                        channels=128,
                        num_elems=CHW,
                        num_idxs=sb,
                    )
                    nc.vector.tensor_tensor(
                        out=mask[:], in0=mask[:], in1=adjt[:], op=mult
                    )
                    nc.sync.dma_start(
                        out=outp[b * 128 : (b + 1) * 128, j * CHW : (j + 1) * CHW],
                        in_=mask[:],
                    )

            # software pipeline: dense phase of block b-1 overlaps MLP of b
            for b in range(NBLK):
                mlp_block(b)
                if b > 0:
                    dense_block(b - 1)
            dense_block(NBLK - 1)

    nc.compile()
    return nc


def kernel(embed, row, col, adj, noise, W1, b1, W2, b2, node_idx):
    from concourse.bass_utils import run_bass_kernel_spmd

    embed = np.ascontiguousarray(np.asarray(embed), dtype=np.float32)
    adj = np.ascontiguousarray(np.asarray(adj), dtype=np.float32)
    W1 = np.ascontiguousarray(np.asarray(W1), dtype=np.float32)
    b1 = np.ascontiguousarray(np.asarray(b1), dtype=np.float32).ravel()
    W2 = np.ascontiguousarray(np.asarray(W2), dtype=np.float32)
    b2f = float(np.asarray(b2, dtype=np.float32).ravel()[0])
    nidx = int(np.asarray(node_idx))

    per_core, orders, meta = _prep_host(row, col, noise, adj, embed)
    nc = _build_program(meta, b2f)

    w1a = np.ascontiguousarray(W1[0:64])
    w1b = np.ascontiguousarray(W1[64:128])
    w1c = np.ascontiguousarray(W1[128:192])
    w2v = W2.reshape(-1)
    w2d = np.zeros((128, 2), np.float32)
    w2d[:64, 0] = w2v
    w2d[64:, 1] = w2v
    wbig = np.zeros((128, 128), np.float32)
    wbig[0:64, 0:64] = w1b  # stream1 other-side
    wbig[64:128, 64:128] = w1a  # stream2 other-side
    common = dict(
        e5=np.ascontiguousarray(embed[nidx].reshape(64, 1)),
        w1af=w1a, w1bf=w1b, w1cf=w1c,
        wbig=np.ascontiguousarray(wbig.astype(BF16)),
        w2d=np.ascontiguousarray(w2d.astype(BF16)),
        b1r=np.ascontiguousarray(b1.reshape(1, 64)),
    )
    in_maps = []
    for k in range(NCORES):
        mcore = dict(per_core[k])
        mcore.update(common)
        in_maps.append(mcore)

    res = run_bass_kernel_spmd(nc, in_maps, list(range(NCORES)))
    kernel.last_exec_time_ns = res.exec_time_ns
    it = getattr(res, "instructions_and_trace", None)
    kernel.last_trace_path = it[1] if it else None

    bp_index = (
        np.repeat(np.arange(NBLK), RPB) * 128 + np.tile(np.arange(RPB), NBLK)
    )
    out = np.empty((N, N), np.float32)
    for k in range(NCORES):
        o = np.asarray(res.results[k]["out"])[:, :N].astype(np.float32)
        out[orders[k] + k * RPC] = o[bp_index]
    return out


kernel.last_exec_time_ns = None
kernel.last_trace_path = None
